# revision 1
# baseline (speedup 1.0000x reference)
"""Trainium2 Bass kernel for nn_Block_44358422233377 (dense transformer block).

Strategy (8 NeuronCores, data parallel over (batch, token-half)):
  core c handles batch b = c//2, query-token half m = c%2 (512 tokens).
  Per core: LN1 + K/V projection over the batch's full 1024 tokens
  (K/V recomputed by the sibling core — no collectives needed), Q only for
  own 512 tokens, all 16 heads of attention for own queries, merged
  (attn_proj @ blk_proj) projection, LN2 + MLP for own 512 tokens.

  All activations live in TRANSPOSED layout [channels(partitions), tokens
  (free)] so every linear layer is a chain of lhsT=weight-block matmuls with
  no on-device transposes. LN / softmax statistics along the partition axis
  are computed with all-ones matmuls on the PE (which also broadcasts them
  across partitions for free). Softmax denominators come from augmenting V
  with a ones-column (row 64 of the PV output = sum of exp scores).

  Weight folding (host, exact): LN gains into the following weight matrix,
  LN biases + linear biases into effective biases, softmax scale into Wq,
  attn_proj+blk_proj merged into one matmul, V bias pushed through softmax
  (rows sum to 1) into the merged-proj bias.

  Matmul operands are bf16 (PE 1 cycle/row), accumulation fp32 in PSUM,
  residual path fp32 end-to-end.

  Hardware constraint shaping the code: every instruction may carry at most
  2 sync waits (walrus codegen limit).  Hence: no mid-kernel SBUF pool
  releases (zone-reuse bombs), single-DMA-per-slot weight streams, bias
  adds on DVE (keeps each consumer's producer set small), and tiny DVE
  "touch" ops after DMAs to absorb their semaphores early.
"""
import sys

sys.path.insert(0, "/opt/trn_rl_repo")

import numpy as np
import ml_dtypes

import concourse.bass as bass
import concourse.bacc as bacc
import concourse.mybir as mybir
import concourse.tile as tile
from concourse.bass import ts
from concourse.bass_utils import run_bass_kernel_spmd

F32 = mybir.dt.float32
BF16 = mybir.dt.bfloat16
AF = mybir.ActivationFunctionType
OP = mybir.AluOpType

P = 128
B, N, C, H = 4, 1024, 1024, 16
HD = C // H          # 64
FF = 4 * C           # 4096
NT = N               # context tokens per core
MT = N // 2          # own (query) tokens per core
SB = MT // 2         # MLP token sub-block (256)
EPS = 1e-6
NCK = C // P         # 8 channel chunks
NFF = FF // P        # 32 ff chunks


def build_module():
    nc = bacc.Bacc("TRN2", target_bir_lowering=False, debug=False)

    xt_d = nc.dram_tensor("xt", [P, NCK * NT], BF16, kind="ExternalInput")
    xmy_d = nc.dram_tensor("xmy", [P, NCK * MT], F32, kind="ExternalInput")
    wqkv_d = nc.dram_tensor("wqkv", [16, P, C], BF16, kind="ExternalInput")
    wv_d = nc.dram_tensor("wv", [2, P, NCK * 512], BF16, kind="ExternalInput")
    wm_d = nc.dram_tensor("wm", [NCK, P, C], BF16, kind="ExternalInput")
    w1_d = nc.dram_tensor("w1", [NFF, P, C], BF16, kind="ExternalInput")
    w2_d = nc.dram_tensor("w2", [NCK, P, FF], BF16, kind="ExternalInput")
    bqk_d = nc.dram_tensor("bqk", [P, 16], F32, kind="ExternalInput")
    bm_d = nc.dram_tensor("bm", [P, NCK], F32, kind="ExternalInput")
    b1_d = nc.dram_tensor("b1", [P, NFF], F32, kind="ExternalInput")
    b2_d = nc.dram_tensor("b2", [P, NCK], F32, kind="ExternalInput")
    out_d = nc.dram_tensor("outT", [P, NCK * MT], F32, kind="ExternalOutput")

    with tile.TileContext(nc) as tc:
        with (
            tc.tile_pool(name="const", bufs=1) as cpool,
            tc.tile_pool(name="persist", bufs=1) as big,
            tc.tile_pool(name="sc", bufs=4) as sc,
            tc.tile_pool(name="sq", bufs=2) as sqp,
            tc.tile_pool(name="tmpb", bufs=2) as tmpp,
            tc.tile_pool(name="wblk", bufs=8) as wblk,
            tc.tile_pool(name="wvs", bufs=2) as wvs,
            tc.tile_pool(name="w2s", bufs=2) as w2s,
            tc.tile_pool(name="xas", bufs=4) as xas,
            tc.tile_pool(name="pt", bufs=3) as ptp,
            tc.tile_pool(name="outts", bufs=1) as outts,
            tc.tile_pool(name="ps", bufs=3, space="PSUM") as psp,
            tc.tile_pool(name="psov", bufs=2, space="PSUM") as psov,
        ):
            # ---- constants / biases ----
            ones128 = cpool.tile([P, P], BF16, tag="ones128")
            nc.vector.memset(ones128[:], 1.0)
            ones1 = cpool.tile([1, HD], BF16, tag="ones1")
            nc.vector.memset(ones1[:], 1.0)
            eps_t = cpool.tile([P, 1], F32, tag="eps")
            nc.vector.memset(eps_t[:], EPS)
            dumv = cpool.tile([1, 8], F32, tag="dumv")
            bqk_t = cpool.tile([P, 16], F32, tag="bqk")
            nc.sync.dma_start(bqk_t[:], bqk_d[:])
            bm_t = cpool.tile([P, NCK], F32, tag="bm")
            nc.sync.dma_start(bm_t[:], bm_d[:])
            b1_t = cpool.tile([P, NFF], F32, tag="b1")
            nc.sync.dma_start(b1_t[:], b1_d[:])
            b2_t = cpool.tile([P, NCK], F32, tag="b2")
            nc.sync.dma_start(b2_t[:], b2_d[:])

            def tdve(ap):
                """Absorb a DMA's semaphore onto the DVE clock."""
                nc.vector.tensor_max(dumv[0:1, 0:1], ap, ap)

            def tpe(ap):
                """Absorb a weight-DMA's semaphore onto the PE clock via a
                tiny throwaway ldweights (next matmul reloads anyway)."""
                nc.tensor.ldweights(ap)

            _ring = {}

            def stream_tile(pool, shape, dtype, tag, name, bufs):
                """Rotating DMA-target tile. All stream rings use bufs=8 ==
                the HWDGE queue round-robin period, so a slot's successive
                DMAs land on the same queue (FIFO) and need no WAW waits —
                instructions may carry at most 2 sync waits."""
                lst = _ring.setdefault(tag, [])
                t = pool.tile(shape, dtype, tag=tag, name=name)
                lst.append(t)
                return t

            # ---- persistent activations ----
            xnT = big.tile([P, NCK, NT], BF16, tag="xnT")
            kT = big.tile([P, NCK, NT], BF16, tag="kt_ht")   # shares slot w/ hT
            qT = big.tile([P, NCK, MT], BF16, tag="qT")
            vE = big.tile([P, NCK, H, HD + 1], BF16, tag="vE")
            oT = big.tile([P, NCK, MT], BF16, tag="ot_x2n")
            x2 = big.tile([P, NCK, MT], F32, tag="x2")
            
            inv1 = big.tile([P, 2, 512], BF16, tag="inv1")
            ngm1 = big.tile([P, 2, 512], BF16, tag="ngm1")
            inv2 = big.tile([P, 512], BF16, tag="inv2")
            ngm2 = big.tile([P, 512], BF16, tag="ngm2")

            nc.vector.memset(vE[:, :, :, HD:HD + 1], 1.0)

            # =============== Phase B: LN1 stats (pass 1) ===============
            pssq = [psp.tile([P, 1024], F32, tag="ps", name=f"pssq{tb}")
                    for tb in range(2)]
            for k in range(NCK):
                xa = stream_tile(xas, [P, NT], BF16, "xa", f"xa{k}", 8)
                nc.sync.dma_start(xa[:], xt_d[:, ts(k, NT)])
                tdve(xa[0:1, 0:1])
                sq = sqp.tile([P, NT], BF16, tag="sq", name=f"sqB{k}")
                nc.vector.tensor_mul(sq[:], xa[:], xa[:])
                for tb in range(2):
                    nc.tensor.matmul(pssq[tb][:, 0:512], ones128[:],
                                     xa[:, ts(tb, 512)],
                                     start=(k == 0), stop=(k == NCK - 1),
                                     skip_group_check=True)
                    nc.tensor.matmul(pssq[tb][:, 512:1024], ones128[:],
                                     sq[:, ts(tb, 512)],
                                     start=(k == 0), stop=(k == NCK - 1),
                                     skip_group_check=True)

            for tb in range(2):
                mu = sc.tile([P, 512], F32, tag="sc", name=f"mu1_{tb}")
                nc.scalar.activation(mu[:], pssq[tb][:, 0:512], AF.Copy,
                                     scale=1.0 / C)
                musq = sc.tile([P, 512], F32, tag="sc", name=f"musq1_{tb}")
                nc.vector.tensor_mul(musq[:], mu[:], mu[:])
                var = sc.tile([P, 512], F32, tag="sc", name=f"var1_{tb}")
                nc.vector.scalar_tensor_tensor(
                    var[:], pssq[tb][:, 512:1024], 1.0 / C, musq[:],
                    op0=OP.mult, op1=OP.subtract)
                std = sc.tile([P, 512], F32, tag="sc", name=f"std1_{tb}")
                nc.scalar.activation(std[:], var[:], AF.Sqrt, bias=eps_t[:])
                with nc.allow_low_precision(reason="ln scale bf16"):
                    nc.vector.reciprocal(inv1[:, tb, :], std[:])
                    nc.vector.scalar_tensor_tensor(
                        ngm1[:, tb, :], mu[:], -1.0, inv1[:, tb, :],
                        op0=OP.mult, op1=OP.mult)

            # =============== LN1 apply (pass 2, re-stream x) ===============
            for k in range(NCK):
                xa = stream_tile(xas, [P, NT], BF16, "xa", f"xb{k}", 8)
                nc.sync.dma_start(xa[:], xt_d[:, ts(k, NT)])
                tdve(xa[0:1, 0:1])
                for tb in range(2):
                    tmp = tmpp.tile([P, 512], BF16, tag="tmpb",
                                    name=f"lt{k}_{tb}")
                    nc.vector.tensor_mul(tmp[:], xa[:, ts(tb, 512)],
                                         inv1[:, tb, :])
                    nc.vector.tensor_add(xnT[:, k, ts(tb, 512)], tmp[:],
                                         ngm1[:, tb, :])

            # =============== Phase C: QKV projections ===============
            # Q (own 512 tokens): pairs of out-chunks share one psum tile
            for op_ in range(4):
                ps = psp.tile([P, 1024], F32, tag="ps", name=f"psq{op_}")
                for half in range(2):
                    o = 2 * op_ + half
                    w = stream_tile(wblk, [P, NCK, P], BF16, "wblk",
                                    f"wq{o}", 8)
                    nc.sync.dma_start(w[:], wqkv_d[o])
                    tpe(w[0:1, 0, 0:1])
                    for k in range(NCK):
                        nc.tensor.matmul(ps[:, ts(half, 512)], w[:, k, :],
                                         xnT[:, k, 0:MT],
                                         start=(k == 0), stop=(k == NCK - 1),
                                         skip_group_check=True)
                for half in range(2):
                    o = 2 * op_ + half
                    nc.vector.tensor_scalar_add(qT[:, o, :],
                                                ps[:, ts(half, 512)],
                                                bqk_t[:, o:o + 1])
            # K (all 1024 tokens)
            for o in range(NCK):
                w = stream_tile(wblk, [P, NCK, P], BF16, "wblk",
                                f"wk{o}", 8)
                nc.sync.dma_start(w[:], wqkv_d[NCK + o])
                tpe(w[0:1, 0, 0:1])
                ps = psp.tile([P, NT], F32, tag="ps", name=f"psk{o}")
                for k in range(NCK):
                    for tb in range(2):
                        nc.tensor.matmul(ps[:, ts(tb, 512)], w[:, k, :],
                                         xnT[:, k, ts(tb, 512)],
                                         start=(k == 0), stop=(k == NCK - 1),
                                         skip_group_check=True)
                nc.vector.tensor_scalar_add(kT[:, o, :], ps[:],
                                            bqk_t[:, NCK + o:NCK + o + 1])
            # V (normal layout [tokens, channels], ones col appended)
            for vb in range(2):
                wv = wvs.tile([P, NCK, 512], BF16, tag="wv",
                              name=f"wv{vb}")
                nc.sync.dma_start(wv[:], wv_d[vb])
                tpe(wv[0:1, 0, 0:1])
                wvt = [wv[:, k, :] for k in range(NCK)]
                for tp in range(4):
                    ps = psp.tile([P, 1024], F32, tag="ps",
                                  name=f"psv{vb}_{tp}")
                    for half in range(2):
                        t8 = 2 * tp + half
                        for k in range(NCK):
                            nc.tensor.matmul(ps[:, ts(half, 512)],
                                             xnT[:, k, ts(t8, P)], wvt[k],
                                             start=(k == 0),
                                             stop=(k == NCK - 1),
                                             skip_group_check=True)
                    for half in range(2):
                        t8 = 2 * tp + half
                        nc.scalar.copy(
                            vE[:, t8, ts(vb, 8), 0:HD],
                            ps[:, ts(half, 512)].rearrange(
                                "p (h d) -> p h d", d=HD))

            # =============== Phase D: attention ===============
            for hp in range(8):
                ovs = [psov.tile([HD + 1, 512], F32, tag="ov",
                                 name=f"ov{hp}_{e}") for e in range(2)]
                for j in range(4):
                    pse = [psp.tile([P, 1024], F32, tag="ps",
                                    name=f"psS{hp}_{j}_{e}")
                           for e in range(2)]
                    for t in range(2):
                        for e in range(2):
                            nk = 2 * j + t
                            hb = e * HD
                            nc.tensor.matmul(
                                pse[e][:, ts(t, 512)],
                                kT[hb:hb + HD, hp, ts(nk, P)],
                                qT[hb:hb + HD, hp, :],
                                start=True, stop=True)
                    for e in range(2):
                        h = 2 * hp + e
                        pt = ptp.tile([P, 1024], BF16, tag=f"pt{e}",
                                      name=f"pt{hp}_{j}_{e}")
                        nc.scalar.activation(pt[:], pse[e][:], AF.Exp)
                        for t in range(2):
                            nk = 2 * j + t
                            nc.tensor.matmul(
                                ovs[e][:], vE[:, nk, h, :],
                                pt[:, ts(t, 512)],
                                start=(j == 0 and t == 0),
                                stop=(j == 3 and t == 1),
                                skip_group_check=True)
                for e in range(2):
                    hb = e * HD
                    rec = sc.tile([1, 512], BF16, tag="rec",
                                  name=f"rec{hp}_{e}")
                    with nc.allow_low_precision(reason="softmax denom bf16"):
                        nc.vector.reciprocal(rec[:], ovs[e][HD:HD + 1, :])
                    bc = psp.tile([P, 1024], F32, tag="ps",
                                  name=f"bc{hp}_{e}")
                    nc.tensor.matmul(bc[0:HD, 0:512], ones1[:], rec[:],
                                     start=True, stop=True)
                    nc.scalar.copy(oT[hb:hb + HD, hp, :],
                                   ovs[e][0:HD, :])
                    nc.vector.tensor_mul(oT[hb:hb + HD, hp, :],
                                         oT[hb:hb + HD, hp, :],
                                         bc[0:HD, 0:512])

            # =============== Phase E: merged proj + residual (fp32) ===============
            # all 8 wm DMAs emitted as one uninterrupted run (queue alignment)
            wm_tiles = []
            for o in range(NCK):
                w = stream_tile(wblk, [P, NCK, P], BF16, "wblk", f"wm{o}", 8)
                nc.sync.dma_start(w[:], wm_d[o])
                tpe(w[0:1, 0, 0:1])
                wm_tiles.append(w)
            xmyt = big.tile([P, NCK, MT], F32, tag="xmyt")
            for o in range(NCK):
                nc.sync.dma_start(xmyt[:, o, :], xmy_d[:, ts(o, 512)])
            tdve(xmyt[0:1, 0, 0:1])
            for op_ in range(4):
                ps = psp.tile([P, 1024], F32, tag="ps", name=f"psE{op_}")
                for half in range(2):
                    o = 2 * op_ + half
                    w = wm_tiles[o]
                    for k in range(NCK):
                        nc.tensor.matmul(ps[:, ts(half, 512)], w[:, k, :],
                                         oT[:, k, :],
                                         start=(k == 0), stop=(k == NCK - 1),
                                         skip_group_check=True)
                for half in range(2):
                    o = 2 * op_ + half
                    nc.vector.scalar_tensor_tensor(
                        x2[:, o, :], ps[:, ts(half, 512)], bm_t[:, o:o + 1],
                        xmyt[:, o, :], op0=OP.add, op1=OP.add)

            # =============== Phase F: LN2 ===============
            # stats accumulate in the (post-attention idle) "ov" psum slots so
            # they don't steal phase E's "ps" rotation while overlapped
            ps2a = psov.tile([P, 512], F32, tag="ov", name="psF_s")
            ps2b = psov.tile([P, 512], F32, tag="ov", name="psF_q")
            for k in range(NCK):
                xb = tmpp.tile([P, 512], BF16, tag="tmpb", name=f"x2b{k}")
                nc.vector.tensor_max(xb[:], x2[:, k, :], x2[:, k, :])
                sq = sqp.tile([P, NT], BF16, tag="sq", name=f"sqF{k}")
                nc.vector.tensor_mul(sq[:, 0:512], xb[:], xb[:])
                nc.tensor.matmul(ps2a[:], ones128[:], xb[:],
                                 start=(k == 0), stop=(k == NCK - 1),
                                 skip_group_check=True)
                nc.tensor.matmul(ps2b[:], ones128[:], sq[:, 0:512],
                                 start=(k == 0), stop=(k == NCK - 1),
                                 skip_group_check=True)
            mu = sc.tile([P, 512], F32, tag="sc", name="mu2")
            nc.scalar.activation(mu[:], ps2a[:], AF.Copy, scale=1.0 / C)
            musq = sc.tile([P, 512], F32, tag="sc", name="musq2")
            nc.vector.tensor_mul(musq[:], mu[:], mu[:])
            var = sc.tile([P, 512], F32, tag="sc", name="var2")
            nc.vector.scalar_tensor_tensor(
                var[:], ps2b[:], 1.0 / C, musq[:],
                op0=OP.mult, op1=OP.subtract)
            std = sc.tile([P, 512], F32, tag="sc", name="std2")
            nc.scalar.activation(std[:], var[:], AF.Sqrt, bias=eps_t[:])
            with nc.allow_low_precision(reason="ln scale bf16"):
                nc.vector.reciprocal(inv2[:], std[:])
                nc.vector.scalar_tensor_tensor(
                    ngm2[:], mu[:], -1.0, inv2[:], op0=OP.mult, op1=OP.mult)
            x2n = big.tile([P, NCK, MT], BF16, tag="ot_x2n", name="x2n")
            for k in range(NCK):
                tmp = tmpp.tile([P, 512], BF16, tag="tmpb", name=f"l2t{k}")
                nc.vector.tensor_mul(tmp[:], x2[:, k, :], inv2[:])
                nc.vector.tensor_add(x2n[:, k, :], tmp[:], ngm2[:])

            # =============== Phase G: fc1 + gelu (full 512 tokens) ===============
            hT = big.tile([P, NFF, MT], BF16, tag="kt_ht", name="hT")
            for fp_ in range(NFF // 2):
                ps = psp.tile([P, 1024], F32, tag="ps", name=f"psG{fp_}")
                for half in range(2):
                    f = 2 * fp_ + half
                    w = stream_tile(wblk, [P, NCK, P], BF16, "wblk",
                                    f"w1_{f}", 8)
                    nc.sync.dma_start(w[:], w1_d[f])
                    tpe(w[0:1, 0, 0:1])
                    for k in range(NCK):
                        nc.tensor.matmul(
                            ps[:, ts(half, 512)], w[:, k, :], x2n[:, k, :],
                            start=(k == 0), stop=(k == NCK - 1),
                            skip_group_check=True)
                for half in range(2):
                    f = 2 * fp_ + half
                    nc.scalar.activation(hT[:, f, :], ps[:, ts(half, 512)],
                                         AF.Gelu, bias=b1_t[:, f:f + 1])

            # =============== Phase H: fc2 + residual (single weight pass) ===============
            for op_ in range(4):
                ps = psp.tile([P, 1024], F32, tag="ps", name=f"psH{op_}")
                for half in range(2):
                    o = 2 * op_ + half
                    w2t = w2s.tile([P, NFF, P], BF16, tag="w2f",
                                   name=f"w2_{o}")
                    nc.sync.dma_start(w2t[:], w2_d[o])
                    tpe(w2t[0:1, 0, 0:1])
                    for f in range(NFF):
                        nc.tensor.matmul(
                            ps[:, ts(half, 512)], w2t[:, f, :], hT[:, f, :],
                            start=(f == 0), stop=(f == NFF - 1),
                            skip_group_check=True)
                for half in range(2):
                    o = 2 * op_ + half
                    outt = outts.tile([P, MT], F32, tag="outt",
                                      name=f"out{o}")
                    nc.vector.scalar_tensor_tensor(
                        outt[:], ps[:, ts(half, 512)], b2_t[:, o:o + 1],
                        x2[:, o, :], op0=OP.add, op1=OP.add)
                    nc.sync.dma_start(out_d[:, ts(o, 512)], outt[:])

    nc.compile()
    return nc


# ---------------- host side ----------------

def _bf16(a):
    return np.ascontiguousarray(a.astype(ml_dtypes.bfloat16))


def _f32(a):
    return np.ascontiguousarray(a.astype(np.float32))


def prepare_inputs(x, qkv_w, qkv_b, attn_proj_w, attn_proj_b, blk_proj_w,
                   blk_proj_b, ln1_g, ln1_b, ln2_g, ln2_b, fc1_w, fc1_b,
                   fc2_w, fc2_b, mask):
    """Fold weights and build per-core input maps."""
    x = np.asarray(x, np.float32)
    qkv_w = np.asarray(qkv_w, np.float64)
    qkv_b = np.asarray(qkv_b, np.float64)
    scale = float(HD) ** -0.5

    g1 = np.asarray(ln1_g, np.float64)
    bl1 = np.asarray(ln1_b, np.float64)
    Wq = qkv_w[0:C] * g1[None, :] * scale
    bq = (qkv_w[0:C] @ bl1 + qkv_b[0:C]) * scale
    Wk = qkv_w[C:2 * C] * g1[None, :]
    bk = qkv_w[C:2 * C] @ bl1 + qkv_b[C:2 * C]
    Wv = qkv_w[2 * C:] * g1[None, :]
    bv = qkv_w[2 * C:] @ bl1 + qkv_b[2 * C:]

    A = np.asarray(attn_proj_w, np.float64)
    Bw = np.asarray(blk_proj_w, np.float64)
    Wm = Bw @ A
    bm = Wm @ bv + Bw @ np.asarray(attn_proj_b, np.float64) \
        + np.asarray(blk_proj_b, np.float64)

    g2 = np.asarray(ln2_g, np.float64)
    bl2 = np.asarray(ln2_b, np.float64)
    W1 = np.asarray(fc1_w, np.float64) * g2[None, :]
    b1 = np.asarray(fc1_w, np.float64) @ bl2 + np.asarray(fc1_b, np.float64)
    W2 = np.asarray(fc2_w, np.float64)
    b2 = np.asarray(fc2_b, np.float64)

    WA = np.vstack([Wq, Wk])                                   # [2048, 1024]
    wqkv = _bf16(WA.reshape(16, P, NCK, P).transpose(0, 3, 2, 1)
                 .reshape(16, P, C))
    wv_l = _bf16(Wv.reshape(2, 512, NCK, P).transpose(0, 3, 2, 1)
                 .reshape(2, P, NCK * 512))
    wm_l = _bf16(Wm.reshape(NCK, P, NCK, P).transpose(0, 3, 2, 1)
                 .reshape(NCK, P, C))
    w1_l = _bf16(W1.reshape(NFF, P, NCK, P).transpose(0, 3, 2, 1)
                 .reshape(NFF, P, C))
    w2_l = _bf16(W2.reshape(NCK, P, NFF, P).transpose(0, 3, 2, 1)
                 .reshape(NCK, P, FF))
    bqk_l = _f32(np.concatenate([bq, bk]).reshape(16, P).T)
    bm_l = _f32(bm.reshape(NCK, P).T)
    b1_l = _f32(b1.reshape(NFF, P).T)
    b2_l = _f32(b2.reshape(NCK, P).T)

    shared = dict(wqkv=wqkv, wv=wv_l, wm=wm_l, w1=w1_l, w2=w2_l,
                  bqk=bqk_l, bm=bm_l, b1=b1_l, b2=b2_l)

    in_maps = []
    for c in range(8):
        b, m = divmod(c, 2)
        xb = x[b]                                              # [1024, 1024]
        xp = np.concatenate([xb[m * MT:(m + 1) * MT],
                             xb[(1 - m) * MT:(2 - m) * MT]], axis=0)
        xt_l = _bf16(xp.reshape(NT, NCK, P).transpose(2, 1, 0)
                     .reshape(P, NCK * NT))
        xmy_l = _f32(xb[m * MT:(m + 1) * MT].reshape(MT, NCK, P)
                     .transpose(2, 1, 0).reshape(P, NCK * MT))
        in_maps.append(dict(shared, xt=xt_l, xmy=xmy_l))
    return in_maps


def gather_output(results):
    out = np.empty((B, N, C), np.float32)
    for c in range(8):
        b, m = divmod(c, 2)
        O = results[c]["outT"].reshape(P, NCK, MT)
        out[b, m * MT:(m + 1) * MT, :] = \
            O.transpose(2, 1, 0).reshape(MT, C)
    return out


_CACHE = {}


def kernel(**inputs):
    if "nc" not in _CACHE:
        _CACHE["nc"] = build_module()
    nc = _CACHE["nc"]
    in_maps = prepare_inputs(**inputs)
    res = run_bass_kernel_spmd(nc, in_maps, core_ids=list(range(8)))
    return gather_output(res.results)



# revision 5
# speedup vs baseline: 1.2253x; 1.2253x over previous
"""Trainium2 Bass kernel for nn_Block_44358422233377 (dense transformer block).

v2: fp8e4m3+DoubleRow attention side (4x cheaper per MAC in the cost model),
bf16 MLP, 4-deep query-block software pipeline overlapping the ACT-bound
softmax-exp with PE-bound MLP-front work, single-pass x streaming, DMA issue
split across SP (x, attn weights, w2, out) and Pool (w1, fc1-psum drains).

Sharding: core c = (batch b = c//2, query-half m = c%2); K/V recomputed per
sibling (no collectives). Activations live transposed [channels(part), tok].

Numerics: attn weights *32 -> e4m3; scores psum = 1024*s_true; softmax via
exp(s_raw/8192 - 2) in e4m3 (denominator from the ones-column of V, common
shift cancels); bc = PE-broadcast of 16/denom; merged-proj descale 1/16384
folded into the x2 write. MLP stays bf16. Output DMA'd bf16, upcast on host.
Measured end-to-end rel err ~7e-3 (budget 2e-2).
"""
import sys

sys.path.insert(0, "/opt/trn_rl_repo")

import numpy as np
import ml_dtypes

import concourse.bass as bass
import concourse.bacc as bacc
import concourse.mybir as mybir
import concourse.tile as tile
from concourse.bass import ts
from concourse.bass_utils import run_bass_kernel_spmd

F32 = mybir.dt.float32
BF16 = mybir.dt.bfloat16
FP8 = mybir.dt.float8e4
AF = mybir.ActivationFunctionType
OP = mybir.AluOpType
DR = mybir.MatmulPerfMode.DoubleRow

P = 128
B, N, C, H = 4, 1024, 1024, 16
HD = C // H          # 64
FF = 4 * C           # 4096
NT = N               # context tokens per core
MT = N // 2          # own (query) tokens per core
QB = 128             # query sub-block (pipeline granularity)
NQB = MT // QB       # 4
EPS = 1e-6
NCK = C // P         # 8 channel chunks
NFF = FF // P        # 32 ff chunks
SW = 32.0            # fp8 weight scale
EXPS = 1.0 / (8.0 * SW * SW)       # exp scale  (= 1/8192)
RECS = 4.0                         # oE scale (vs o_true: SW*RECS)
OESUB = 512.0                      # staging scale: oE_pre = ov/OESUB
PROJS = 1.0 / (SW * SW * RECS)     # proj psum descale (= 1/16384)


DBG = {}


def build_module():
    nc = bacc.Bacc("TRN2", target_bir_lowering=False, debug=False)

    xt_d = nc.dram_tensor("xt", [P, NCK * NT], BF16, kind="ExternalInput")
    wq_d = nc.dram_tensor("wq", [NCK, P, C], FP8, kind="ExternalInput")
    wk_d = nc.dram_tensor("wk", [NCK, P, C], FP8, kind="ExternalInput")
    wv_d = nc.dram_tensor("wv", [2, 4, P, 1024], FP8, kind="ExternalInput")
    wm_d = nc.dram_tensor("wm", [NCK, P, C], FP8, kind="ExternalInput")
    w1_d = nc.dram_tensor("w1", [NFF, P, C], BF16, kind="ExternalInput")
    w2_d = nc.dram_tensor("w2", [NCK, P, FF], BF16, kind="ExternalInput")
    bqk_d = nc.dram_tensor("bqk", [P, 16], F32, kind="ExternalInput")
    bm_d = nc.dram_tensor("bm", [P, NCK], F32, kind="ExternalInput")
    b1_d = nc.dram_tensor("b1", [P, NFF], F32, kind="ExternalInput")
    b2_d = nc.dram_tensor("b2", [P, NCK], F32, kind="ExternalInput")
    out_d = nc.dram_tensor("outT", [P, NCK * MT], BF16, kind="ExternalOutput")

    wv_tiles = {}
    wm_tiles = {}
    ln2_ps = {}
    ln2_sc = {}
    ov_hold = {}

    with tile.TileContext(nc) as tc:
        with (
            tc.tile_pool(name="const", bufs=1) as cpool,
            tc.tile_pool(name="persist", bufs=1) as big,
            tc.tile_pool(name="sc", bufs=4) as sc,
            tc.tile_pool(name="sq", bufs=2) as sqp,
            tc.tile_pool(name="tmpb", bufs=2) as tmpp,
            tc.tile_pool(name="x2t", bufs=2) as x2tp,
            tc.tile_pool(name="ln2", bufs=4) as ln2p,
            tc.tile_pool(name="ln2s", bufs=2) as ln2sp,
            tc.tile_pool(name="wblk", bufs=16) as wblk,
            tc.tile_pool(name="w2s", bufs=2) as w2s,
            tc.tile_pool(name="pt", bufs=3) as ptp,
            tc.tile_pool(name="rc", bufs=2) as rcp,
            tc.tile_pool(name="outts", bufs=2) as outts,
            tc.tile_pool(name="psA", bufs=2, space="PSUM") as psA,
            tc.tile_pool(name="psF", bufs=1, space="PSUM") as psF,
            tc.tile_pool(name="psO", bufs=3, space="PSUM") as psO,
        ):
            # ---- constants / biases ----
            ones128 = cpool.tile([P, P], BF16, tag="ones128")
            nc.vector.memset(ones128[:], 1.0)
            ones64 = cpool.tile([1, HD], BF16, tag="ones64")
            nc.vector.memset(ones64[:], RECS * OESUB)
            eps_t = cpool.tile([P, 1], F32, tag="eps")
            nc.vector.memset(eps_t[:], EPS)
            nm2_t = cpool.tile([P, 1], F32, tag="nm2")
            nc.vector.memset(nm2_t[:], -2.0)
            dumv = cpool.tile([1, 8], F32, tag="dumv")
            bqk_t = cpool.tile([P, 16], F32, tag="bqk")
            bm_t = cpool.tile([P, NCK], F32, tag="bm")
            b1_t = cpool.tile([P, NFF], F32, tag="b1")
            b2_t = cpool.tile([P, NCK], F32, tag="b2")

            def tdve(ap):
                """Absorb a DMA's semaphore onto the DVE clock."""
                nc.vector.tensor_max(dumv[0:1, 0:1], ap, ap)

            def tpe(ap):
                """Absorb a weight-DMA's semaphore onto the PE clock."""
                nc.tensor.ldweights(ap)

            # ---- persistent activations ----
            xt = big.tile([P, NCK, NT], BF16, tag="xt")
            xnT = big.tile([P, NCK, NT], FP8, tag="xnT")
            kE = big.tile([P, 2, 4, NT], FP8, tag="kE")
            qE = big.tile([P, 2, 4, MT], FP8, tag="qE")
            vE = big.tile([P, 4, 2, H, HD + 1], FP8, tag="vE")
            oE = big.tile([P, NCK, MT], FP8, tag="oE")
            x2 = big.tile([P, NCK, MT], BF16, tag="x2")
            x2n = big.tile([P, NCK, MT], BF16, tag="x2n")
            h1T = big.tile([P, NFF, MT], BF16, tag="h1T")
            w1R = big.tile([P, NFF, NCK, P], BF16, tag="w1R")

            inv1 = big.tile([P, 2, 512], BF16, tag="inv1")
            ngm1 = big.tile([P, 2, 512], BF16, tag="ngm1")
            DBG.update(xnT=xnT, kE=kE, qE=qE, vE=vE, oE=oE, x2=x2,
                       x2n=x2n, h1T=h1T, inv1=inv1, ngm1=ngm1)

            nc.vector.memset(vE[:, :, :, :, HD:HD + 1], 1.0)

            # wblk ring slot plan (16 bufs): wk 0-7, wq 8-15, wv 0-7 (after
            # K chains), wm 8-15 (after Q chains) — no cross-stream cycles.
            # wk tiles allocated first (ring order); DMAs issued on Pool
            # after the xt stream so LN1 stats aren't delayed.
            wk_tiles = {}
            for c in range(NCK):
                wk_tiles[c] = wblk.tile([P, 4, 2, P], FP8, tag="wblk",
                                        name=f"wk{c}")

            # =============== LN1 stats (single x pass) ===============
            pssq = [psA.tile([P, 1024], F32, tag="ps", name=f"pssq{tb}")
                    for tb in range(2)]
            for k in range(NCK):
                if k % 2 == 0:
                    nc.sync.dma_start(xt[:, k, :], xt_d[:, ts(k, NT)])
                else:
                    nc.gpsimd.dma_start(xt[:, k, :], xt_d[:, ts(k, NT)])
                tdve(xt[0:1, k, 0:1])
                for tb in range(2):
                    sq = sqp.tile([P, 512], BF16, tag="sq",
                                  name=f"sqB{k}_{tb}")
                    nc.vector.tensor_mul(sq[:], xt[:, k, ts(tb, 512)],
                                         xt[:, k, ts(tb, 512)])
                    nc.tensor.matmul(pssq[tb][:, 0:512], ones128[:],
                                     xt[:, k, ts(tb, 512)],
                                     start=(k == 0), stop=(k == NCK - 1),
                                     skip_group_check=True)
                    nc.tensor.matmul(pssq[tb][:, 512:1024], ones128[:],
                                     sq[:],
                                     start=(k == 0), stop=(k == NCK - 1),
                                     skip_group_check=True)

            # weight/bias DMA issue, after xt so stats aren't stalled
            for c in range(NCK):
                nc.gpsimd.dma_start(wk_tiles[c][:], wk_d[c])
                tpe(wk_tiles[c][0:1, 0, 0, 0:1])
            nc.sync.dma_start(bqk_t[:], bqk_d[:])
            nc.sync.dma_start(bm_t[:], bm_d[:])
            nc.sync.dma_start(b1_t[:], b1_d[:])
            nc.sync.dma_start(b2_t[:], b2_d[:])

            for tb in range(2):
                mu = sc.tile([P, 512], BF16, tag="scb", name=f"mu1_{tb}")
                with nc.allow_low_precision(reason="ln stats bf16"):
                    nc.scalar.activation(mu[:], pssq[tb][:, 0:512], AF.Copy,
                                         scale=1.0 / C)
                musq = sc.tile([P, 512], BF16, tag="scb", name=f"musq1_{tb}")
                nc.vector.tensor_mul(musq[:], mu[:], mu[:])
                var = sc.tile([P, 512], BF16, tag="scb", name=f"var1_{tb}")
                with nc.allow_low_precision(reason="ln stats bf16"):
                    nc.vector.scalar_tensor_tensor(
                        var[:], pssq[tb][:, 512:1024], 1.0 / C, musq[:],
                        op0=OP.mult, op1=OP.subtract)
                std = sc.tile([P, 512], BF16, tag="scb", name=f"std1_{tb}")
                nc.scalar.activation(std[:], var[:], AF.Sqrt, bias=eps_t[:])
                with nc.allow_low_precision(reason="ln scale bf16"):
                    nc.vector.reciprocal(inv1[:, tb, :], std[:])
                    nc.vector.scalar_tensor_tensor(
                        ngm1[:, tb, :], mu[:], -1.0, inv1[:, tb, :],
                        op0=OP.mult, op1=OP.mult)

            # =============== LN1 apply (from SBUF) -> xnT fp8 ===============
            for k in range(NCK):
                for tb in range(2):
                    tmp = tmpp.tile([P, 512], BF16, tag="tmpb",
                                    name=f"lt{k}_{tb}")
                    nc.vector.tensor_mul(tmp[:], xt[:, k, ts(tb, 512)],
                                         inv1[:, tb, :])
                    with nc.allow_low_precision(reason="fp8 activations"):
                        nc.vector.tensor_add(xnT[:, k, ts(tb, 512)], tmp[:],
                                             ngm1[:, tb, :])

            # =============== Q / K projections (DoubleRow fp8) ===============
            # chunk c = lohi*4 + hg holds perm'd out-channels (see host prep)
            def qk_chain(ps_slice, w, qsl):
                for t in range(4):
                    nc.tensor.matmul(ps_slice, w[:, t, :, :],
                                     xnT[:, 2 * t:2 * t + 2, qsl],
                                     start=(t == 0), stop=(t == 3),
                                     perf_mode=DR, skip_group_check=True)

            for i in range(4):
                ps = psA.tile([P, 1024], F32, tag="ps", name=f"psq{i}")
                for half in range(2):
                    c = 2 * i + half
                    w = wblk.tile([P, 4, 2, P], FP8, tag="wblk",
                                  name=f"wq{c}")
                    nc.sync.dma_start(w[:], wq_d[c])
                    tpe(w[0:1, 0, 0, 0:1])
                    qk_chain(ps[:, ts(half, 512)], w, slice(0, MT))
                for half in range(2):
                    c = 2 * i + half
                    lohi, hg = c // 4, c % 4
                    with nc.allow_low_precision(reason="fp8 activations"):
                        nc.scalar.activation(
                            qE[:, lohi, hg, :], ps[:, ts(half, 512)],
                            AF.Identity, bias=bqk_t[:, c:c + 1])
            # wv on Pool (ring slots 0-7, reusing wk slots after K chains)
            for vb in range(2):
                for t in range(4):
                    w = wblk.tile([P, 2, 512], FP8, tag="wblk",
                                  name=f"wv{vb}_{t}")
                    nc.gpsimd.dma_start(w[:], wv_d[vb, t])
                    tpe(w[0:1, 0, 0:1])
                    wv_tiles[(vb, t)] = w
            # wm upfront on SP (slots 8-15 after wq), then w1 resident on SP
            for o in range(NCK):
                w = wblk.tile([P, 4, 2, P], FP8, tag="wblk", name=f"wm{o}")
                nc.sync.dma_start(w[:], wm_d[o])
                tpe(w[0:1, 0, 0, 0:1])
                wm_tiles[o] = w
            for f in range(NFF):
                nc.sync.dma_start(w1R[:, f, :, :], w1_d[f])
            tdve(w1R[0:1, 0, 0, 0:1])
            # K chains ordered so head-group hg's chunks (hg, hg+4) finish
            # first, letting window-0 scores start while K still runs
            for c in [0, 4, 1, 5, 2, 6, 3, 7]:
                ps = psA.tile([P, NT], F32, tag="ps", name=f"psk{c}")
                for tb in range(2):
                    qk_chain(ps[:, ts(tb, 512)], wk_tiles[c],
                             slice(tb * 512, tb * 512 + 512))
                lohi, hg = c // 4, c % 4
                with nc.allow_low_precision(reason="fp8 activations"):
                    nc.scalar.activation(kE[:, lohi, hg, :], ps[:],
                                         AF.Identity,
                                         bias=bqk_t[:, NCK + c:NCK + c + 1])

            # =============== V projection (DoubleRow fp8) ===============
            # out [128 tok, 512 vd] per (tok-chunk t8, vb); vE gets v_hat=32v
            for t8 in range(NCK):
                ps = psA.tile([P, 1024], F32, tag="ps", name=f"psv{t8}")
                for vb in range(2):
                    for t in range(4):
                        nc.tensor.matmul(
                            ps[:, ts(vb, 512)],
                            xnT[:, 2 * t:2 * t + 2, ts(t8, P)],
                            wv_tiles[(vb, t)][:],
                            start=(t == 0), stop=(t == 3),
                            perf_mode=DR, skip_group_check=True)
                jg, pr = t8 // 2, t8 % 2
                for vb in range(2):
                    # spread the drain ops over DVE and ACT so neither
                    # serial queue gates the first PV (GPSIMD can't read
                    # PSUM per the BIR verifier)
                    with nc.allow_low_precision(reason="fp8 acts"):
                        if (2 * t8 + vb) % 2 == 0:
                            nc.vector.tensor_scalar_mul(
                                vE[:, jg, pr, ts(vb, 8), 0:HD],
                                ps[:, ts(vb, 512)].rearrange(
                                    "p (h d) -> p h d", d=HD), 1.0)
                        else:
                            nc.scalar.copy(
                                vE[:, jg, pr, ts(vb, 8), 0:HD],
                                ps[:, ts(vb, 512)].rearrange(
                                    "p (h d) -> p h d", d=HD))

            # =============== pipelined attention + MLP-front ===============
            def mlpa_thunks(qb):
                """proj+LN2+fc1 work units for query block qb (deps in
                order); emitted interleaved with attention of block qb+1."""
                th = []

                def proj_half(hf):
                    def f():
                        ps = psF.tile([P, 4, QB], F32, tag="pf",
                                      name=f"pm{qb}_{hf}")
                        for o in range(4 * hf, 4 * hf + 4):
                            wt = wm_tiles[o]
                            for t in range(4):
                                nc.tensor.matmul(
                                    ps[:, o - 4 * hf, :], wt[:, t, :, :],
                                    oE[:, 2 * t:2 * t + 2, ts(qb, QB)],
                                    start=(t == 0), stop=(t == 3),
                                    perf_mode=DR, skip_group_check=True)
                        for o in range(4 * hf, 4 * hf + 4):
                            t_ = x2tp.tile([P, QB], BF16, tag="x2t",
                                           name=f"x2t{qb}_{o}")
                            nc.vector.tensor_scalar(
                                t_[:], ps[:, o - 4 * hf, :], PROJS,
                                bm_t[:, o:o + 1], op0=OP.mult, op1=OP.add)
                            with nc.allow_low_precision(reason="x2 bf16"):
                                nc.vector.tensor_add(
                                    x2[:, o, ts(qb, QB)], t_[:],
                                    xt[:, o, qb * QB:qb * QB + QB])
                    return f
                th.append(proj_half(0))
                th.append(proj_half(1))

                def ln2_stats():
                    # sequential chains (sq first, then x): interleaved
                    # chains in one 2KB zero region corrupt each other via
                    # pending-zero re-marking; sequential chains are safe
                    psa = psO.tile([P, 4, QB], F32, tag="ov",
                                   name=f"pl2_{qb}")
                    ln2_ps[qb] = psa
                    for k in range(NCK):
                        sq2 = sqp.tile([P, QB], BF16, tag="sq",
                                       name=f"sq2_{qb}_{k}")
                        nc.vector.tensor_mul(sq2[:], x2[:, k, ts(qb, QB)],
                                             x2[:, k, ts(qb, QB)])
                        nc.tensor.matmul(psa[:, 1, :], ones128[:], sq2[:],
                                         start=(k == 0), stop=(k == NCK - 1),
                                         skip_group_check=True)
                    for k in range(NCK):
                        nc.tensor.matmul(psa[:, 0, :], ones128[:],
                                         x2[:, k, ts(qb, QB)],
                                         start=(k == 0), stop=(k == NCK - 1),
                                         skip_group_check=True)
                th.append(ln2_stats)

                def ln2_fin():
                    psa = ln2_ps.pop(qb)
                    psb = psa[:, 1:2, :]
                    mu = ln2p.tile([P, QB], F32, tag="l2", name=f"mu2_{qb}")
                    nc.scalar.activation(mu[:], psa[:, 0, :], AF.Copy,
                                         scale=1.0 / C)
                    musq = ln2p.tile([P, QB], F32, tag="l2",
                                     name=f"msq2_{qb}")
                    nc.vector.tensor_mul(musq[:], mu[:], mu[:])
                    var = ln2p.tile([P, QB], F32, tag="l2", name=f"var2_{qb}")
                    nc.vector.scalar_tensor_tensor(
                        var[:], psb[:, 0, :], 1.0 / C, musq[:],
                        op0=OP.mult, op1=OP.subtract)
                    # inv-std = exp(-0.5*ln(var+eps)): Ln and Exp share an
                    # ACT table, so no table switch amid the exp stream
                    lv = ln2p.tile([P, QB], F32, tag="l2", name=f"lv2_{qb}")
                    nc.scalar.activation(lv[:], var[:], AF.Ln, bias=eps_t[:])
                    iv = ln2sp.tile([P, QB], BF16, tag="iv2",
                                    name=f"iv2_{qb}")
                    ng = ln2sp.tile([P, QB], BF16, tag="ng2",
                                    name=f"ng2_{qb}")
                    with nc.allow_low_precision(reason="ln scale bf16"):
                        nc.scalar.activation(iv[:], lv[:], AF.Exp, scale=-0.5)
                        nc.vector.scalar_tensor_tensor(
                            ng[:], mu[:], -1.0, iv[:],
                            op0=OP.mult, op1=OP.mult)
                    ln2_sc[qb] = (iv, ng)
                    DBG[f"iv2_{qb}"] = iv
                    DBG[f"ng2_{qb}"] = ng
                    DBG[f"mu2_{qb}"] = mu
                    DBG[f"var2_{qb}"] = var
                th.append(ln2_fin)

                def x2n_w(half):
                    def f():
                        iv, ng = ln2_sc[qb]
                        for k in range(4 * half, 4 * half + 4):
                            t_ = x2tp.tile([P, QB], BF16, tag="x2t",
                                           name=f"xnt{qb}_{k}")
                            nc.vector.tensor_mul(t_[:], x2[:, k, ts(qb, QB)],
                                                 iv[:])
                            with nc.allow_low_precision(reason="fp8 acts"):
                                nc.vector.tensor_add(x2n[:, k, ts(qb, QB)],
                                                     t_[:], ng[:])
                    return f
                th.append(x2n_w(0))
                th.append(x2n_w(1))

                def fc1_grp(g):
                    def f():
                        ps = psF.tile([P, 4, QB], F32, tag="pf",
                                      name=f"p1_{qb}_{g}")
                        for fi in range(4):
                            fch = 4 * g + fi
                            for k in range(NCK):
                                nc.tensor.matmul(
                                    ps[:, fi, :], w1R[:, fch, k, :],
                                    x2n[:, k, ts(qb, QB)],
                                    start=(k == 0), stop=(k == NCK - 1),
                                    skip_group_check=True)
                        # drain psum -> h1T (gelu deferred to tail);
                        # GPSIMD can't read PSUM, so DVE does it
                        nc.vector.tensor_scalar_mul(
                            h1T[:, 4 * g:4 * g + 4, ts(qb, QB)], ps[:], 1.0)
                        if qb == NQB - 1:
                            # last block: gelu chases fc1 so fc2 can stream
                            for fi in range(4):
                                fch = 4 * g + fi
                                nc.scalar.activation(
                                    h1T[:, fch, :], h1T[:, fch, :], AF.Gelu,
                                    bias=b1_t[:, fch:fch + 1])
                    return f
                f1 = [fc1_grp(g) for g in range(NFF // 4)]
                return th, f1

            pend = []
            fc1s = {}
            for qb in range(NQB):
                for h in range(H):
                    b_, hg, e = h % 4, h // 4, h % 2
                    sp = psA.tile([P, NCK, QB], F32, tag="ps",
                                  name=f"sp{qb}_{h}")
                    for kc in range(NCK):
                        nc.tensor.matmul(
                            sp[:, kc, :],
                            kE[32 * b_:32 * b_ + 32, :, hg, ts(kc, P)],
                            qE[32 * b_:32 * b_ + 32, :, hg, ts(qb, QB)],
                            start=True, stop=True, perf_mode=DR,
                            skip_group_check=True,
                            tile_position=(32 * b_, 0))
                    pt = ptp.tile([P, NCK, QB], FP8, tag="pt",
                                  name=f"pt{qb}_{h}")
                    with nc.allow_low_precision(reason="fp8 exp scores"):
                        nc.scalar.activation(pt[:], sp[:], AF.Exp,
                                             bias=nm2_t[:], scale=EXPS)
                    ov = psO.tile([HD + 1, QB], F32, tag="ov",
                                  name=f"ov{qb}_{h}")
                    for a in range(4):
                        nc.tensor.matmul(ov[:], vE[:, a, :, h, :],
                                         pt[:, 2 * a:2 * a + 2, :],
                                         start=(a == 0), stop=(a == 3),
                                         perf_mode=DR, skip_group_check=True)
                    rc = rcp.tile([1, QB], BF16, tag="rc",
                                  name=f"rc{qb}_{h}")
                    with nc.allow_low_precision(reason="softmax denom bf16"):
                        nc.vector.reciprocal(rc[:], ov[64:65, :])
                    # stage ov into oE (SBUF) first: walrus allows only one
                    # PSUM input per DVE op, so the bc multiply is in-place
                    ch = h // 2
                    with nc.allow_low_precision(reason="fp8 oE"):
                        if e == 0:
                            nc.vector.tensor_scalar_mul(
                                oE[0:HD, ch, ts(qb, QB)], ov[0:HD, :],
                                1.0 / OESUB)
                        else:
                            nc.scalar.mul(oE[HD:P, ch, ts(qb, QB)],
                                          ov[0:HD, :], 1.0 / OESUB)
                    if e == 0:
                        ov_hold[0] = rc
                    else:
                        rc0 = ov_hold.pop(0)
                        bcp = psO.tile([P, QB], F32, tag="ov",
                                       name=f"bc{qb}_{ch}")
                        nc.tensor.matmul(bcp[0:HD, :], ones64[:], rc0[:],
                                         start=True, stop=True,
                                         skip_group_check=True)
                        nc.tensor.matmul(bcp[HD:P, :], ones64[:], rc[:],
                                         start=True, stop=True,
                                         skip_group_check=True)
                        with nc.allow_low_precision(reason="fp8 oE"):
                            nc.vector.tensor_mul(
                                oE[0:HD, ch, ts(qb, QB)],
                                oE[0:HD, ch, ts(qb, QB)], bcp[0:HD, :])
                            nc.vector.tensor_mul(
                                oE[HD:P, ch, ts(qb, QB)],
                                oE[HD:P, ch, ts(qb, QB)], bcp[HD:P, :])
                    # interleave one pending MLP unit per head slot
                    if pend:
                        pend.pop(0)()
                for t_ in pend:
                    t_()
                fr, f1 = mlpa_thunks(qb)
                # window qb+1 runs front(qb) plus fc1(qb-1): the serial
                # proj->LN2->x2n chain gets a full window of attention
                # cover before its fc1 consumes it one window later
                pend = fr + fc1s.get(qb - 1, [])
                fc1s[qb] = f1
            for t_ in pend:
                t_()
            for t_ in fc1s[NQB - 1]:
                t_()

            # =============== tail: fc2 (gelu already chased fc1) ===============
            for i in range(4):
                ps = psA.tile([P, 1024], F32, tag="ps", name=f"psf2_{i}")
                for half in range(2):
                    o = 2 * i + half
                    for fh in range(2):
                        w2t = w2s.tile([P, NFF // 2, P], BF16, tag="w2f",
                                       name=f"w2_{o}_{fh}")
                        nc.sync.dma_start(
                            w2t[:], w2_d[o][:, fh * 2048:(fh + 1) * 2048])
                        tpe(w2t[0:1, 0, 0:1])
                        for fi in range(NFF // 2):
                            f = fh * (NFF // 2) + fi
                            nc.tensor.matmul(
                                ps[:, ts(half, 512)], w2t[:, fi, :],
                                h1T[:, f, :],
                                start=(f == 0), stop=(f == NFF - 1),
                                skip_group_check=True)
                for half in range(2):
                    o = 2 * i + half
                    outt = outts.tile([P, MT], BF16, tag="outt",
                                      name=f"out{o}")
                    with nc.allow_low_precision(reason="bf16 output"):
                        nc.vector.scalar_tensor_tensor(
                            outt[:], ps[:, ts(half, 512)], b2_t[:, o:o + 1],
                            x2[:, o, :], op0=OP.add, op1=OP.add)
                    if o % 2 == 0:
                        nc.sync.dma_start(out_d[:, ts(o, 512)], outt[:])
                    else:
                        nc.gpsimd.dma_start(out_d[:, ts(o, 512)], outt[:])

    nc.compile()
    return nc


# ---------------- host side ----------------

def _bf16(a):
    return np.ascontiguousarray(a.astype(ml_dtypes.bfloat16))


def _f32(a):
    return np.ascontiguousarray(a.astype(np.float32))


def _fp8(a):
    return np.ascontiguousarray(
        np.clip(a, -240.0, 240.0).astype(ml_dtypes.float8_e4m3))


def _qk_perm():
    """out-channel permutation: chunk c = lohi*4+hg, partition p = b*32+r
    holds orig channel 64*(4*hg+b) + 32*lohi + r."""
    perm = np.empty(C, np.int64)
    for c in range(NCK):
        lohi, hg = c // 4, c % 4
        for p in range(P):
            b_, r = p // 32, p % 32
            perm[c * P + p] = 64 * (4 * hg + b_) + 32 * lohi + r
    return perm


def _dr_pack(W):
    """[out (nck*128), in C] -> [nck, P, (t, j, m)] DoubleRow layout:
    element [c][p][t, j, m] = W[c*128+m, (2t+j)*128+p]."""
    nck = W.shape[0] // P
    Wr = W.reshape(nck, P, NCK, P)          # [c, m, kin, p]
    out = np.empty((nck, P, 4, 2, P), W.dtype)
    for t in range(4):
        for j in range(2):
            out[:, :, t, j, :] = Wr[:, :, 2 * t + j, :].transpose(0, 2, 1)
    return out.reshape(nck, P, C)


def prepare_inputs(x, qkv_w, qkv_b, attn_proj_w, attn_proj_b, blk_proj_w,
                   blk_proj_b, ln1_g, ln1_b, ln2_g, ln2_b, fc1_w, fc1_b,
                   fc2_w, fc2_b, mask):
    x = np.asarray(x, np.float32)
    qkv_w = np.asarray(qkv_w, np.float64)
    qkv_b = np.asarray(qkv_b, np.float64)

    g1 = np.asarray(ln1_g, np.float64)
    bl1 = np.asarray(ln1_b, np.float64)
    Wq = qkv_w[0:C] * g1[None, :]
    bq = qkv_w[0:C] @ bl1 + qkv_b[0:C]
    Wk = qkv_w[C:2 * C] * g1[None, :]
    bk = qkv_w[C:2 * C] @ bl1 + qkv_b[C:2 * C]
    Wv = qkv_w[2 * C:] * g1[None, :]
    bv = qkv_w[2 * C:] @ bl1 + qkv_b[2 * C:]

    A = np.asarray(attn_proj_w, np.float64)
    Bw = np.asarray(blk_proj_w, np.float64)
    Wm = Bw @ A
    bm = Wm @ bv + Bw @ np.asarray(attn_proj_b, np.float64) \
        + np.asarray(blk_proj_b, np.float64)

    g2 = np.asarray(ln2_g, np.float64)
    bl2 = np.asarray(ln2_b, np.float64)
    W1 = np.asarray(fc1_w, np.float64) * g2[None, :]
    b1 = np.asarray(fc1_w, np.float64) @ bl2 + np.asarray(fc1_b, np.float64)
    W2 = np.asarray(fc2_w, np.float64)
    b2 = np.asarray(fc2_b, np.float64)

    perm = _qk_perm()
    wq_l = _fp8(_dr_pack((SW * Wq)[perm]))
    wk_l = _fp8(_dr_pack((SW * Wk)[perm]))
    bqP = (SW * bq)[perm]
    bkP = (SW * bk)[perm]
    # V: [vb][t][p][(j, n)]: SW * Wv[vb*512+n, (2t+j)*128+p]
    WvS = (SW * Wv).reshape(2, 512, NCK, P)     # [vb, n, kin, p]
    wv_l = np.empty((2, 4, P, 2, 512), np.float64)
    for t in range(4):
        for j in range(2):
            wv_l[:, t, :, j, :] = WvS[:, :, 2 * t + j, :].transpose(0, 2, 1)
    wv_l = _fp8(wv_l.reshape(2, 4, P, 1024))
    wm_l = _fp8(_dr_pack(SW * Wm))
    w1_l = _bf16(W1.reshape(NFF, P, NCK, P).transpose(0, 3, 2, 1)
                 .reshape(NFF, P, C))
    w2_l = _bf16(W2.reshape(NCK, P, NFF, P).transpose(0, 3, 2, 1)
                 .reshape(NCK, P, FF))
    bqk_l = _f32(np.concatenate([bqP.reshape(NCK, P).T,
                                 bkP.reshape(NCK, P).T], axis=1))
    bm_l = _f32(bm.reshape(NCK, P).T)
    b1_l = _f32(b1.reshape(NFF, P).T)
    b2_l = _f32(b2.reshape(NCK, P).T)

    shared = dict(wq=wq_l, wk=wk_l, wv=wv_l, wm=wm_l, w1=w1_l, w2=w2_l,
                  bqk=bqk_l, bm=bm_l, b1=b1_l, b2=b2_l)

    in_maps = []
    for core in range(8):
        b_, m = divmod(core, 2)
        xb = x[b_]
        xp = np.concatenate([xb[m * MT:(m + 1) * MT],
                             xb[(1 - m) * MT:(2 - m) * MT]], axis=0)
        xt_l = _bf16(xp.reshape(NT, NCK, P).transpose(2, 1, 0)
                     .reshape(P, NCK * NT))
        in_maps.append(dict(shared, xt=xt_l))
    return in_maps


def gather_output(results):
    out = np.empty((B, N, C), np.float32)
    for core in range(8):
        b_, m = divmod(core, 2)
        O = np.asarray(results[core]["outT"]).astype(np.float32)
        O = O.reshape(P, NCK, MT)
        out[b_, m * MT:(m + 1) * MT, :] = O.transpose(2, 1, 0).reshape(MT, C)
    return out


_CACHE = {}


def kernel(**inputs):
    if "nc" not in _CACHE:
        _CACHE["nc"] = build_module()
    nc = _CACHE["nc"]
    in_maps = prepare_inputs(**inputs)
    res = run_bass_kernel_spmd(nc, in_maps, core_ids=list(range(8)))
    return gather_output(res.results)


# revision 6
# speedup vs baseline: 1.2514x; 1.0214x over previous
"""Trainium2 Bass kernel for nn_Block_44358422233377 (dense transformer block).

v2: fp8e4m3+DoubleRow attention side (4x cheaper per MAC in the cost model),
bf16 MLP, 4-deep query-block software pipeline overlapping the ACT-bound
softmax-exp with PE-bound MLP-front work, single-pass x streaming, DMA issue
split across SP (x, attn weights, w2, out) and Pool (w1, fc1-psum drains).

Sharding: core c = (batch b = c//2, query-half m = c%2); K/V recomputed per
sibling (no collectives). Activations live transposed [channels(part), tok].

Numerics: attn weights *32 -> e4m3; scores psum = 1024*s_true; softmax via
exp(s_raw/8192 - 2) in e4m3 (denominator from the ones-column of V, common
shift cancels); bc = PE-broadcast of 16/denom; merged-proj descale 1/16384
folded into the x2 write. MLP stays bf16. Output DMA'd bf16, upcast on host.
Measured end-to-end rel err ~7e-3 (budget 2e-2).
"""
import sys

sys.path.insert(0, "/opt/trn_rl_repo")

import numpy as np
import ml_dtypes

import concourse.bass as bass
import concourse.bacc as bacc
import concourse.mybir as mybir
import concourse.tile as tile
from concourse.bass import ts
from concourse.bass_utils import run_bass_kernel_spmd

F32 = mybir.dt.float32
BF16 = mybir.dt.bfloat16
FP8 = mybir.dt.float8e4
AF = mybir.ActivationFunctionType
OP = mybir.AluOpType
DR = mybir.MatmulPerfMode.DoubleRow

P = 128
B, N, C, H = 4, 1024, 1024, 16
HD = C // H          # 64
FF = 4 * C           # 4096
NT = N               # context tokens per core
MT = N // 2          # own (query) tokens per core
QB = 128             # query sub-block (pipeline granularity)
NQB = MT // QB       # 4
EPS = 1e-6
NCK = C // P         # 8 channel chunks
NFF = FF // P        # 32 ff chunks
SW = 32.0            # fp8 weight scale
EXPS = 1.0 / (8.0 * SW * SW)       # exp scale  (= 1/8192)
RECS = 4.0                         # oE scale (vs o_true: SW*RECS)
OESUB = 512.0                      # staging scale: oE_pre = ov/OESUB
PROJS = 1.0 / (SW * SW * RECS)     # proj psum descale (= 1/16384)


DBG = {}


def build_module():
    nc = bacc.Bacc("TRN2", target_bir_lowering=False, debug=False)

    xt_d = nc.dram_tensor("xt", [P, NCK * NT], BF16, kind="ExternalInput")
    wq_d = nc.dram_tensor("wq", [NCK, P, C], FP8, kind="ExternalInput")
    wk_d = nc.dram_tensor("wk", [NCK, P, C], FP8, kind="ExternalInput")
    wv_d = nc.dram_tensor("wv", [2, 4, P, 1024], FP8, kind="ExternalInput")
    wm_d = nc.dram_tensor("wm", [NCK, P, C], FP8, kind="ExternalInput")
    w1_d = nc.dram_tensor("w1", [NFF, P, C], BF16, kind="ExternalInput")
    w2_d = nc.dram_tensor("w2", [NCK, P, FF], BF16, kind="ExternalInput")
    bqk_d = nc.dram_tensor("bqk", [P, 16], F32, kind="ExternalInput")
    bm_d = nc.dram_tensor("bm", [P, NCK], F32, kind="ExternalInput")
    b1_d = nc.dram_tensor("b1", [P, NFF], F32, kind="ExternalInput")
    b2_d = nc.dram_tensor("b2", [P, NCK], F32, kind="ExternalInput")
    out_d = nc.dram_tensor("outT", [P, NCK * MT], BF16, kind="ExternalOutput")

    wv_tiles = {}
    wm_tiles = {}
    ln2_ps = {}
    ln2_sc = {}
    ov_hold = {}

    with tile.TileContext(nc) as tc:
        with (
            tc.tile_pool(name="const", bufs=1) as cpool,
            tc.tile_pool(name="persist", bufs=1) as big,
            tc.tile_pool(name="sc", bufs=4) as sc,
            tc.tile_pool(name="sq", bufs=2) as sqp,
            tc.tile_pool(name="tmpb", bufs=2) as tmpp,
            tc.tile_pool(name="x2t", bufs=2) as x2tp,
            tc.tile_pool(name="ln2", bufs=4) as ln2p,
            tc.tile_pool(name="ln2s", bufs=2) as ln2sp,
            tc.tile_pool(name="wblk", bufs=16) as wblk,
            tc.tile_pool(name="w2s", bufs=2) as w2s,
            tc.tile_pool(name="pt", bufs=3) as ptp,
            tc.tile_pool(name="rc", bufs=2) as rcp,
            tc.tile_pool(name="outts", bufs=2) as outts,
            tc.tile_pool(name="psA", bufs=2, space="PSUM") as psA,
            tc.tile_pool(name="psF", bufs=1, space="PSUM") as psF,
            tc.tile_pool(name="psO", bufs=3, space="PSUM") as psO,
        ):
            # ---- constants / biases ----
            ones128 = cpool.tile([P, P], BF16, tag="ones128")
            nc.vector.memset(ones128[:], 1.0)
            ones64 = cpool.tile([1, HD], BF16, tag="ones64")
            nc.vector.memset(ones64[:], RECS * OESUB)
            eps_t = cpool.tile([P, 1], F32, tag="eps")
            nc.vector.memset(eps_t[:], EPS)
            nm2_t = cpool.tile([P, 1], F32, tag="nm2")
            nc.vector.memset(nm2_t[:], -2.0)
            dumv = cpool.tile([1, 8], F32, tag="dumv")
            bqk_t = cpool.tile([P, 16], F32, tag="bqk")
            bm_t = cpool.tile([P, NCK], F32, tag="bm")
            b1_t = cpool.tile([P, NFF], F32, tag="b1")
            b2_t = cpool.tile([P, NCK], F32, tag="b2")

            def tdve(ap):
                """Absorb a DMA's semaphore onto the DVE clock."""
                nc.vector.tensor_max(dumv[0:1, 0:1], ap, ap)

            def tpe(ap):
                """Absorb a weight-DMA's semaphore onto the PE clock."""
                nc.tensor.ldweights(ap)

            # ---- persistent activations ----
            xt = big.tile([P, NCK, NT], BF16, tag="xt")
            xnT = big.tile([P, NCK, NT], FP8, tag="xnT")
            kE = big.tile([P, 2, 4, NT], FP8, tag="kE")
            qE = big.tile([P, 2, 4, MT], FP8, tag="qE")
            vE = big.tile([P, 4, 2, H, HD + 1], FP8, tag="vE")
            oE = big.tile([P, NCK, MT], FP8, tag="oE")
            x2 = big.tile([P, NCK, MT], BF16, tag="x2")
            x2n = big.tile([P, NCK, MT], BF16, tag="x2n")
            h1T = big.tile([P, NFF, MT], BF16, tag="h1T")
            w1R = big.tile([P, NFF, NCK, P], BF16, tag="w1R")

            inv1 = big.tile([P, 2, 512], BF16, tag="inv1")
            ngm1 = big.tile([P, 2, 512], BF16, tag="ngm1")
            DBG.update(xnT=xnT, kE=kE, qE=qE, vE=vE, oE=oE, x2=x2,
                       x2n=x2n, h1T=h1T, inv1=inv1, ngm1=ngm1)

            nc.vector.memset(vE[:, :, :, :, HD:HD + 1], 1.0)

            # wblk ring slot plan (16 bufs): wk 0-7, wq 8-15, wv 0-7 (after
            # K chains), wm 8-15 (after Q chains) — no cross-stream cycles.
            # wk tiles allocated first (ring order); DMAs issued on Pool
            # after the xt stream so LN1 stats aren't delayed.
            wk_tiles = {}
            for c in range(NCK):
                wk_tiles[c] = wblk.tile([P, 4, 2, P], FP8, tag="wblk",
                                        name=f"wk{c}")

            # =============== LN1 stats (single x pass) ===============
            pssq = [psA.tile([P, 1024], F32, tag="ps", name=f"pssq{tb}")
                    for tb in range(2)]
            for k in range(NCK):
                if k % 2 == 0:
                    nc.sync.dma_start(xt[:, k, :], xt_d[:, ts(k, NT)])
                else:
                    nc.gpsimd.dma_start(xt[:, k, :], xt_d[:, ts(k, NT)])
                tdve(xt[0:1, k, 0:1])
                for tb in range(2):
                    sq = sqp.tile([P, 512], BF16, tag="sq",
                                  name=f"sqB{k}_{tb}")
                    nc.vector.tensor_mul(sq[:], xt[:, k, ts(tb, 512)],
                                         xt[:, k, ts(tb, 512)])
                    nc.tensor.matmul(pssq[tb][:, 0:512], ones128[:],
                                     xt[:, k, ts(tb, 512)],
                                     start=(k == 0), stop=(k == NCK - 1),
                                     skip_group_check=True)
                    nc.tensor.matmul(pssq[tb][:, 512:1024], ones128[:],
                                     sq[:],
                                     start=(k == 0), stop=(k == NCK - 1),
                                     skip_group_check=True)

            # weight/bias DMA issue, after xt so stats aren't stalled
            for c in range(NCK):
                nc.gpsimd.dma_start(wk_tiles[c][:], wk_d[c])
                tpe(wk_tiles[c][0:1, 0, 0, 0:1])
            nc.sync.dma_start(bqk_t[:], bqk_d[:])
            nc.sync.dma_start(bm_t[:], bm_d[:])
            nc.sync.dma_start(b1_t[:], b1_d[:])
            nc.sync.dma_start(b2_t[:], b2_d[:])

            for tb in range(2):
                mu = sc.tile([P, 512], BF16, tag="scb", name=f"mu1_{tb}")
                with nc.allow_low_precision(reason="ln stats bf16"):
                    nc.scalar.activation(mu[:], pssq[tb][:, 0:512], AF.Copy,
                                         scale=1.0 / C)
                musq = sc.tile([P, 512], BF16, tag="scb", name=f"musq1_{tb}")
                nc.vector.tensor_mul(musq[:], mu[:], mu[:])
                var = sc.tile([P, 512], BF16, tag="scb", name=f"var1_{tb}")
                with nc.allow_low_precision(reason="ln stats bf16"):
                    nc.vector.scalar_tensor_tensor(
                        var[:], pssq[tb][:, 512:1024], 1.0 / C, musq[:],
                        op0=OP.mult, op1=OP.subtract)
                std = sc.tile([P, 512], BF16, tag="scb", name=f"std1_{tb}")
                nc.scalar.activation(std[:], var[:], AF.Sqrt, bias=eps_t[:])
                with nc.allow_low_precision(reason="ln scale bf16"):
                    nc.vector.reciprocal(inv1[:, tb, :], std[:])
                    nc.vector.scalar_tensor_tensor(
                        ngm1[:, tb, :], mu[:], -1.0, inv1[:, tb, :],
                        op0=OP.mult, op1=OP.mult)

            # =============== LN1 apply (from SBUF) -> xnT fp8 ===============
            for k in range(NCK):
                for tb in range(2):
                    tmp = tmpp.tile([P, 512], BF16, tag="tmpb",
                                    name=f"lt{k}_{tb}")
                    nc.vector.tensor_mul(tmp[:], xt[:, k, ts(tb, 512)],
                                         inv1[:, tb, :])
                    with nc.allow_low_precision(reason="fp8 activations"):
                        nc.vector.tensor_add(xnT[:, k, ts(tb, 512)], tmp[:],
                                             ngm1[:, tb, :])

            # =============== Q / K projections (DoubleRow fp8) ===============
            # chunk c = lohi*4 + hg holds perm'd out-channels (see host prep)
            def qk_chain(ps_slice, w, qsl):
                for t in range(4):
                    nc.tensor.matmul(ps_slice, w[:, t, :, :],
                                     xnT[:, 2 * t:2 * t + 2, qsl],
                                     start=(t == 0), stop=(t == 3),
                                     perf_mode=DR, skip_group_check=True)

            for i in range(4):
                ps = psA.tile([P, 1024], F32, tag="ps", name=f"psq{i}")
                for half in range(2):
                    c = 2 * i + half
                    w = wblk.tile([P, 4, 2, P], FP8, tag="wblk",
                                  name=f"wq{c}")
                    nc.sync.dma_start(w[:], wq_d[c])
                    tpe(w[0:1, 0, 0, 0:1])
                    qk_chain(ps[:, ts(half, 512)], w, slice(0, MT))
                for half in range(2):
                    c = 2 * i + half
                    lohi, hg = c // 4, c % 4
                    with nc.allow_low_precision(reason="fp8 activations"):
                        nc.scalar.activation(
                            qE[:, lohi, hg, :], ps[:, ts(half, 512)],
                            AF.Identity, bias=bqk_t[:, c:c + 1])
            # wv on Pool (ring slots 0-7, reusing wk slots after K chains)
            for vb in range(2):
                for t in range(4):
                    w = wblk.tile([P, 2, 512], FP8, tag="wblk",
                                  name=f"wv{vb}_{t}")
                    nc.gpsimd.dma_start(w[:], wv_d[vb, t])
                    tpe(w[0:1, 0, 0:1])
                    wv_tiles[(vb, t)] = w
            # wm upfront on SP (slots 8-15 after wq), then w1 resident on SP
            for o in range(NCK):
                w = wblk.tile([P, 4, 2, P], FP8, tag="wblk", name=f"wm{o}")
                nc.sync.dma_start(w[:], wm_d[o])
                tpe(w[0:1, 0, 0, 0:1])
                wm_tiles[o] = w
            for f in range(NFF):
                nc.sync.dma_start(w1R[:, f, :, :], w1_d[f])
            tdve(w1R[0:1, 0, 0, 0:1])
            # K chains ordered so head-group hg's chunks (hg, hg+4) finish
            # first, letting window-0 scores start while K still runs
            for c in [0, 4, 1, 5, 2, 6, 3, 7]:
                ps = psA.tile([P, NT], F32, tag="ps", name=f"psk{c}")
                for tb in range(2):
                    qk_chain(ps[:, ts(tb, 512)], wk_tiles[c],
                             slice(tb * 512, tb * 512 + 512))
                lohi, hg = c // 4, c % 4
                with nc.allow_low_precision(reason="fp8 activations"):
                    nc.scalar.activation(kE[:, lohi, hg, :], ps[:],
                                         AF.Identity,
                                         bias=bqk_t[:, NCK + c:NCK + c + 1])

            # =============== V projection (DoubleRow fp8) ===============
            # out [128 tok, 512 vd] per (tok-chunk t8, vb); vE gets v_hat=32v
            for t8 in range(NCK):
                ps = psA.tile([P, 1024], F32, tag="ps", name=f"psv{t8}")
                for vb in range(2):
                    for t in range(4):
                        nc.tensor.matmul(
                            ps[:, ts(vb, 512)],
                            xnT[:, 2 * t:2 * t + 2, ts(t8, P)],
                            wv_tiles[(vb, t)][:],
                            start=(t == 0), stop=(t == 3),
                            perf_mode=DR, skip_group_check=True)
                jg, pr = t8 // 2, t8 % 2
                for vb in range(2):
                    # spread the drain ops over DVE and ACT so neither
                    # serial queue gates the first PV (GPSIMD can't read
                    # PSUM per the BIR verifier)
                    with nc.allow_low_precision(reason="fp8 acts"):
                        if (2 * t8 + vb) % 2 == 0:
                            nc.vector.tensor_scalar_mul(
                                vE[:, jg, pr, ts(vb, 8), 0:HD],
                                ps[:, ts(vb, 512)].rearrange(
                                    "p (h d) -> p h d", d=HD), 1.0)
                        else:
                            nc.scalar.copy(
                                vE[:, jg, pr, ts(vb, 8), 0:HD],
                                ps[:, ts(vb, 512)].rearrange(
                                    "p (h d) -> p h d", d=HD))

            # =============== pipelined attention + MLP-front ===============
            def mlpa_thunks(qb):
                """proj+LN2+fc1 work units for query block qb (deps in
                order); emitted interleaved with attention of block qb+1."""
                th = []

                def proj_half(hf):
                    def f():
                        ps = psF.tile([P, 4, QB], F32, tag="pf",
                                      name=f"pm{qb}_{hf}")
                        for o in range(4 * hf, 4 * hf + 4):
                            wt = wm_tiles[o]
                            for t in range(4):
                                nc.tensor.matmul(
                                    ps[:, o - 4 * hf, :], wt[:, t, :, :],
                                    oE[:, 2 * t:2 * t + 2, ts(qb, QB)],
                                    start=(t == 0), stop=(t == 3),
                                    perf_mode=DR, skip_group_check=True)
                        for o in range(4 * hf, 4 * hf + 4):
                            t_ = x2tp.tile([P, QB], BF16, tag="x2t",
                                           name=f"x2t{qb}_{o}")
                            nc.vector.tensor_scalar(
                                t_[:], ps[:, o - 4 * hf, :], PROJS,
                                bm_t[:, o:o + 1], op0=OP.mult, op1=OP.add)
                            with nc.allow_low_precision(reason="x2 bf16"):
                                nc.vector.tensor_add(
                                    x2[:, o, ts(qb, QB)], t_[:],
                                    xt[:, o, qb * QB:qb * QB + QB])
                    return f
                th.append(proj_half(0))
                th.append(proj_half(1))

                def ln2_stats():
                    # sequential chains (sq first, then x): interleaved
                    # chains in one 2KB zero region corrupt each other via
                    # pending-zero re-marking; sequential chains are safe
                    psa = psO.tile([P, 4, QB], F32, tag="ov",
                                   name=f"pl2_{qb}")
                    ln2_ps[qb] = psa
                    for k in range(NCK):
                        sq2 = sqp.tile([P, QB], BF16, tag="sq",
                                       name=f"sq2_{qb}_{k}")
                        nc.gpsimd.tensor_mul(sq2[:], x2[:, k, ts(qb, QB)],
                                             x2[:, k, ts(qb, QB)])
                        nc.tensor.matmul(psa[:, 1, :], ones128[:], sq2[:],
                                         start=(k == 0), stop=(k == NCK - 1),
                                         skip_group_check=True)
                    for k in range(NCK):
                        nc.tensor.matmul(psa[:, 0, :], ones128[:],
                                         x2[:, k, ts(qb, QB)],
                                         start=(k == 0), stop=(k == NCK - 1),
                                         skip_group_check=True)
                th.append(ln2_stats)

                def ln2_fin():
                    psa = ln2_ps.pop(qb)
                    psb = psa[:, 1:2, :]
                    mu = ln2p.tile([P, QB], F32, tag="l2", name=f"mu2_{qb}")
                    nc.scalar.activation(mu[:], psa[:, 0, :], AF.Copy,
                                         scale=1.0 / C)
                    musq = ln2p.tile([P, QB], F32, tag="l2",
                                     name=f"msq2_{qb}")
                    nc.vector.tensor_mul(musq[:], mu[:], mu[:])
                    var = ln2p.tile([P, QB], F32, tag="l2", name=f"var2_{qb}")
                    nc.vector.scalar_tensor_tensor(
                        var[:], psb[:, 0, :], 1.0 / C, musq[:],
                        op0=OP.mult, op1=OP.subtract)
                    # inv-std = exp(-0.5*ln(var+eps)): Ln and Exp share an
                    # ACT table, so no table switch amid the exp stream
                    lv = ln2p.tile([P, QB], F32, tag="l2", name=f"lv2_{qb}")
                    nc.scalar.activation(lv[:], var[:], AF.Ln, bias=eps_t[:])
                    iv = ln2sp.tile([P, QB], BF16, tag="iv2",
                                    name=f"iv2_{qb}")
                    ng = ln2sp.tile([P, QB], BF16, tag="ng2",
                                    name=f"ng2_{qb}")
                    with nc.allow_low_precision(reason="ln scale bf16"):
                        nc.scalar.activation(iv[:], lv[:], AF.Exp, scale=-0.5)
                        nc.vector.scalar_tensor_tensor(
                            ng[:], mu[:], -1.0, iv[:],
                            op0=OP.mult, op1=OP.mult)
                    ln2_sc[qb] = (iv, ng)
                    DBG[f"iv2_{qb}"] = iv
                    DBG[f"ng2_{qb}"] = ng
                    DBG[f"mu2_{qb}"] = mu
                    DBG[f"var2_{qb}"] = var
                th.append(ln2_fin)

                def x2n_w(half):
                    def f():
                        iv, ng = ln2_sc[qb]
                        for k in range(4 * half, 4 * half + 4):
                            t_ = x2tp.tile([P, QB], BF16, tag="x2t",
                                           name=f"xnt{qb}_{k}")
                            nc.gpsimd.tensor_mul(t_[:], x2[:, k, ts(qb, QB)],
                                                 iv[:])
                            with nc.allow_low_precision(reason="x2n bf16"):
                                nc.gpsimd.tensor_add(x2n[:, k, ts(qb, QB)],
                                                     t_[:], ng[:])
                    return f
                th.append(x2n_w(0))
                th.append(x2n_w(1))

                def fc1_grp(g):
                    def f():
                        ps = psF.tile([P, 4, QB], F32, tag="pf",
                                      name=f"p1_{qb}_{g}")
                        for fi in range(4):
                            fch = 4 * g + fi
                            for k in range(NCK):
                                nc.tensor.matmul(
                                    ps[:, fi, :], w1R[:, fch, k, :],
                                    x2n[:, k, ts(qb, QB)],
                                    start=(k == 0), stop=(k == NCK - 1),
                                    skip_group_check=True)
                        # drain psum -> h1T (gelu deferred to tail);
                        # GPSIMD can't read PSUM: alternate DVE/ACT
                        if g % 2 == 0:
                            nc.vector.tensor_scalar_mul(
                                h1T[:, 4 * g:4 * g + 4, ts(qb, QB)],
                                ps[:], 1.0)
                        else:
                            nc.scalar.copy(
                                h1T[:, 4 * g:4 * g + 4, ts(qb, QB)], ps[:])
                        if qb == NQB - 1:
                            # last block: gelu chases fc1 so fc2 can stream
                            for fi in range(4):
                                fch = 4 * g + fi
                                nc.scalar.activation(
                                    h1T[:, fch, :], h1T[:, fch, :], AF.Gelu,
                                    bias=b1_t[:, fch:fch + 1])
                    return f
                f1 = [fc1_grp(g) for g in range(NFF // 4)]
                return th, f1

            pend = []
            fc1s = {}
            for qb in range(NQB):
                for h in range(H):
                    b_, hg, e = h % 4, h // 4, h % 2
                    sp = psA.tile([P, NCK, QB], F32, tag="ps",
                                  name=f"sp{qb}_{h}")
                    for kc in range(NCK):
                        nc.tensor.matmul(
                            sp[:, kc, :],
                            kE[32 * b_:32 * b_ + 32, :, hg, ts(kc, P)],
                            qE[32 * b_:32 * b_ + 32, :, hg, ts(qb, QB)],
                            start=True, stop=True, perf_mode=DR,
                            skip_group_check=True,
                            tile_position=(32 * b_, 0))
                    pt = ptp.tile([P, NCK, QB], FP8, tag="pt",
                                  name=f"pt{qb}_{h}")
                    with nc.allow_low_precision(reason="fp8 exp scores"):
                        nc.scalar.activation(pt[:], sp[:], AF.Exp,
                                             bias=nm2_t[:], scale=EXPS)
                    ov = psO.tile([HD + 1, QB], F32, tag="ov",
                                  name=f"ov{qb}_{h}")
                    for a in range(4):
                        nc.tensor.matmul(ov[:], vE[:, a, :, h, :],
                                         pt[:, 2 * a:2 * a + 2, :],
                                         start=(a == 0), stop=(a == 3),
                                         perf_mode=DR, skip_group_check=True)
                    rc = rcp.tile([1, QB], BF16, tag="rc",
                                  name=f"rc{qb}_{h}")
                    with nc.allow_low_precision(reason="softmax denom bf16"):
                        nc.vector.reciprocal(rc[:], ov[64:65, :])
                    # stage ov into oE (SBUF) first: walrus allows only one
                    # PSUM input per DVE op, so the bc multiply is in-place
                    ch = h // 2
                    with nc.allow_low_precision(reason="fp8 oE"):
                        if e == 0:
                            nc.vector.tensor_scalar_mul(
                                oE[0:HD, ch, ts(qb, QB)], ov[0:HD, :],
                                1.0 / OESUB)
                        else:
                            nc.scalar.mul(oE[HD:P, ch, ts(qb, QB)],
                                          ov[0:HD, :], 1.0 / OESUB)
                    if e == 0:
                        ov_hold[0] = rc
                    else:
                        rc0 = ov_hold.pop(0)
                        bcp = psO.tile([P, QB], F32, tag="ov",
                                       name=f"bc{qb}_{ch}")
                        nc.tensor.matmul(bcp[0:HD, :], ones64[:], rc0[:],
                                         start=True, stop=True,
                                         skip_group_check=True)
                        nc.tensor.matmul(bcp[HD:P, :], ones64[:], rc[:],
                                         start=True, stop=True,
                                         skip_group_check=True)
                        with nc.allow_low_precision(reason="fp8 oE"):
                            nc.vector.tensor_mul(
                                oE[0:HD, ch, ts(qb, QB)],
                                oE[0:HD, ch, ts(qb, QB)], bcp[0:HD, :])
                            nc.vector.tensor_mul(
                                oE[HD:P, ch, ts(qb, QB)],
                                oE[HD:P, ch, ts(qb, QB)], bcp[HD:P, :])
                    # interleave one pending MLP unit per head slot
                    if pend:
                        pend.pop(0)()
                for t_ in pend:
                    t_()
                fr, f1 = mlpa_thunks(qb)
                # window qb+1 runs front(qb) plus fc1(qb-1): the serial
                # proj->LN2->x2n chain gets a full window of attention
                # cover before its fc1 consumes it one window later
                pend = fr + fc1s.get(qb - 1, [])
                fc1s[qb] = f1
            for t_ in pend:
                t_()
            for t_ in fc1s[NQB - 1]:
                t_()

            # =============== tail: fc2 (gelu already chased fc1) ===============
            for i in range(4):
                ps = psA.tile([P, 1024], F32, tag="ps", name=f"psf2_{i}")
                for half in range(2):
                    o = 2 * i + half
                    for fh in range(2):
                        w2t = w2s.tile([P, NFF // 2, P], BF16, tag="w2f",
                                       name=f"w2_{o}_{fh}")
                        nc.sync.dma_start(
                            w2t[:], w2_d[o][:, fh * 2048:(fh + 1) * 2048])
                        tpe(w2t[0:1, 0, 0:1])
                        for fi in range(NFF // 2):
                            f = fh * (NFF // 2) + fi
                            nc.tensor.matmul(
                                ps[:, ts(half, 512)], w2t[:, fi, :],
                                h1T[:, f, :],
                                start=(f == 0), stop=(f == NFF - 1),
                                skip_group_check=True)
                for half in range(2):
                    o = 2 * i + half
                    outt = outts.tile([P, MT], BF16, tag="outt",
                                      name=f"out{o}")
                    with nc.allow_low_precision(reason="bf16 output"):
                        nc.vector.scalar_tensor_tensor(
                            outt[:], ps[:, ts(half, 512)], b2_t[:, o:o + 1],
                            x2[:, o, :], op0=OP.add, op1=OP.add)
                    if o % 2 == 0:
                        nc.sync.dma_start(out_d[:, ts(o, 512)], outt[:])
                    else:
                        nc.gpsimd.dma_start(out_d[:, ts(o, 512)], outt[:])

    nc.compile()
    return nc


# ---------------- host side ----------------

def _bf16(a):
    return np.ascontiguousarray(a.astype(ml_dtypes.bfloat16))


def _f32(a):
    return np.ascontiguousarray(a.astype(np.float32))


def _fp8(a):
    return np.ascontiguousarray(
        np.clip(a, -240.0, 240.0).astype(ml_dtypes.float8_e4m3))


def _qk_perm():
    """out-channel permutation: chunk c = lohi*4+hg, partition p = b*32+r
    holds orig channel 64*(4*hg+b) + 32*lohi + r."""
    perm = np.empty(C, np.int64)
    for c in range(NCK):
        lohi, hg = c // 4, c % 4
        for p in range(P):
            b_, r = p // 32, p % 32
            perm[c * P + p] = 64 * (4 * hg + b_) + 32 * lohi + r
    return perm


def _dr_pack(W):
    """[out (nck*128), in C] -> [nck, P, (t, j, m)] DoubleRow layout:
    element [c][p][t, j, m] = W[c*128+m, (2t+j)*128+p]."""
    nck = W.shape[0] // P
    Wr = W.reshape(nck, P, NCK, P)          # [c, m, kin, p]
    out = np.empty((nck, P, 4, 2, P), W.dtype)
    for t in range(4):
        for j in range(2):
            out[:, :, t, j, :] = Wr[:, :, 2 * t + j, :].transpose(0, 2, 1)
    return out.reshape(nck, P, C)


def prepare_inputs(x, qkv_w, qkv_b, attn_proj_w, attn_proj_b, blk_proj_w,
                   blk_proj_b, ln1_g, ln1_b, ln2_g, ln2_b, fc1_w, fc1_b,
                   fc2_w, fc2_b, mask):
    x = np.asarray(x, np.float32)
    qkv_w = np.asarray(qkv_w, np.float64)
    qkv_b = np.asarray(qkv_b, np.float64)

    g1 = np.asarray(ln1_g, np.float64)
    bl1 = np.asarray(ln1_b, np.float64)
    Wq = qkv_w[0:C] * g1[None, :]
    bq = qkv_w[0:C] @ bl1 + qkv_b[0:C]
    Wk = qkv_w[C:2 * C] * g1[None, :]
    bk = qkv_w[C:2 * C] @ bl1 + qkv_b[C:2 * C]
    Wv = qkv_w[2 * C:] * g1[None, :]
    bv = qkv_w[2 * C:] @ bl1 + qkv_b[2 * C:]

    A = np.asarray(attn_proj_w, np.float64)
    Bw = np.asarray(blk_proj_w, np.float64)
    Wm = Bw @ A
    bm = Wm @ bv + Bw @ np.asarray(attn_proj_b, np.float64) \
        + np.asarray(blk_proj_b, np.float64)

    g2 = np.asarray(ln2_g, np.float64)
    bl2 = np.asarray(ln2_b, np.float64)
    W1 = np.asarray(fc1_w, np.float64) * g2[None, :]
    b1 = np.asarray(fc1_w, np.float64) @ bl2 + np.asarray(fc1_b, np.float64)
    W2 = np.asarray(fc2_w, np.float64)
    b2 = np.asarray(fc2_b, np.float64)

    perm = _qk_perm()
    wq_l = _fp8(_dr_pack((SW * Wq)[perm]))
    wk_l = _fp8(_dr_pack((SW * Wk)[perm]))
    bqP = (SW * bq)[perm]
    bkP = (SW * bk)[perm]
    # V: [vb][t][p][(j, n)]: SW * Wv[vb*512+n, (2t+j)*128+p]
    WvS = (SW * Wv).reshape(2, 512, NCK, P)     # [vb, n, kin, p]
    wv_l = np.empty((2, 4, P, 2, 512), np.float64)
    for t in range(4):
        for j in range(2):
            wv_l[:, t, :, j, :] = WvS[:, :, 2 * t + j, :].transpose(0, 2, 1)
    wv_l = _fp8(wv_l.reshape(2, 4, P, 1024))
    wm_l = _fp8(_dr_pack(SW * Wm))
    w1_l = _bf16(W1.reshape(NFF, P, NCK, P).transpose(0, 3, 2, 1)
                 .reshape(NFF, P, C))
    w2_l = _bf16(W2.reshape(NCK, P, NFF, P).transpose(0, 3, 2, 1)
                 .reshape(NCK, P, FF))
    bqk_l = _f32(np.concatenate([bqP.reshape(NCK, P).T,
                                 bkP.reshape(NCK, P).T], axis=1))
    bm_l = _f32(bm.reshape(NCK, P).T)
    b1_l = _f32(b1.reshape(NFF, P).T)
    b2_l = _f32(b2.reshape(NCK, P).T)

    shared = dict(wq=wq_l, wk=wk_l, wv=wv_l, wm=wm_l, w1=w1_l, w2=w2_l,
                  bqk=bqk_l, bm=bm_l, b1=b1_l, b2=b2_l)

    in_maps = []
    for core in range(8):
        b_, m = divmod(core, 2)
        xb = x[b_]
        xp = np.concatenate([xb[m * MT:(m + 1) * MT],
                             xb[(1 - m) * MT:(2 - m) * MT]], axis=0)
        xt_l = _bf16(xp.reshape(NT, NCK, P).transpose(2, 1, 0)
                     .reshape(P, NCK * NT))
        in_maps.append(dict(shared, xt=xt_l))
    return in_maps


def gather_output(results):
    out = np.empty((B, N, C), np.float32)
    for core in range(8):
        b_, m = divmod(core, 2)
        O = np.asarray(results[core]["outT"]).astype(np.float32)
        O = O.reshape(P, NCK, MT)
        out[b_, m * MT:(m + 1) * MT, :] = O.transpose(2, 1, 0).reshape(MT, C)
    return out


_CACHE = {}


def kernel(**inputs):
    if "nc" not in _CACHE:
        _CACHE["nc"] = build_module()
    nc = _CACHE["nc"]
    in_maps = prepare_inputs(**inputs)
    res = run_bass_kernel_spmd(nc, in_maps, core_ids=list(range(8)))
    return gather_output(res.results)


# revision 11
# speedup vs baseline: 1.2535x; 1.0016x over previous
"""Trainium2 Bass kernel for nn_Block_44358422233377 (dense transformer block).

v2: fp8e4m3+DoubleRow attention side (4x cheaper per MAC in the cost model),
bf16 MLP, 4-deep query-block software pipeline overlapping the ACT-bound
softmax-exp with PE-bound MLP-front work, single-pass x streaming, DMA issue
split across SP (x, attn weights, w2, out) and Pool (w1, fc1-psum drains).

Sharding: core c = (batch b = c//2, query-half m = c%2); K/V recomputed per
sibling (no collectives). Activations live transposed [channels(part), tok].

Numerics: attn weights *32 -> e4m3; scores psum = 1024*s_true; softmax via
exp(s_raw/8192 - 2) in e4m3 (denominator via the ones-column of V; the
common shift cancels in the normalize). oE staged as ov/512 in fp8, then
scaled in place by the PE-broadcast 2048*RECS'/denom; merged-proj descale
1/(SW*SW*RECS) folded into the x2 write. MLP stays bf16 (fp8 fails the
2e-2 gate). Output DMA'd bf16, upcast on host. End-to-end rel err 7.4e-3.

Walrus BIR rules honored (the CoreSim-only version broke all three):
GPSIMD never touches PSUM; DVE/ACT ops read at most one PSUM operand;
two SBUF inputs of a DVE op share a partition base. Interleaved matmul
accumulation chains never share a 2KB PSUM zero region (pending-zero
re-marking silently zeroes the neighbor chain's partials).
"""
import sys

sys.path.insert(0, "/opt/trn_rl_repo")

import numpy as np
import ml_dtypes

import concourse.bass as bass
import concourse.bacc as bacc
import concourse.mybir as mybir
import concourse.tile as tile
from concourse.bass import ts
from concourse.bass_utils import run_bass_kernel_spmd

F32 = mybir.dt.float32
BF16 = mybir.dt.bfloat16
FP8 = mybir.dt.float8e4
AF = mybir.ActivationFunctionType
OP = mybir.AluOpType
DR = mybir.MatmulPerfMode.DoubleRow

P = 128
B, N, C, H = 4, 1024, 1024, 16
HD = C // H          # 64
FF = 4 * C           # 4096
NT = N               # context tokens per core
MT = N // 2          # own (query) tokens per core
QB = 128             # query sub-block (pipeline granularity)
NQB = MT // QB       # 4
EPS = 1e-6
NCK = C // P         # 8 channel chunks
NFF = FF // P        # 32 ff chunks
SW = 32.0            # fp8 weight scale
EXPS = 1.0 / (8.0 * SW * SW)       # exp scale  (= 1/8192)
RECS = 4.0                         # oE scale (vs o_true: SW*RECS)
OESUB = 512.0                      # staging scale: oE_pre = ov/OESUB
PROJS = 1.0 / (SW * SW * RECS)     # proj psum descale (= 1/16384)


DBG = {}


def build_module():
    nc = bacc.Bacc("TRN2", target_bir_lowering=False, debug=False)

    xt_d = nc.dram_tensor("xt", [P, NCK * NT], BF16, kind="ExternalInput")
    wq_d = nc.dram_tensor("wq", [NCK, P, C], FP8, kind="ExternalInput")
    wk_d = nc.dram_tensor("wk", [NCK, P, C], FP8, kind="ExternalInput")
    wv_d = nc.dram_tensor("wv", [2, 4, P, 1024], FP8, kind="ExternalInput")
    wm_d = nc.dram_tensor("wm", [NCK, P, C], FP8, kind="ExternalInput")
    w1_d = nc.dram_tensor("w1", [NFF, P, C], BF16, kind="ExternalInput")
    w2_d = nc.dram_tensor("w2", [NCK, P, FF], BF16, kind="ExternalInput")
    bqk_d = nc.dram_tensor("bqk", [P, 16], F32, kind="ExternalInput")
    bm_d = nc.dram_tensor("bm", [P, NCK], F32, kind="ExternalInput")
    b1_d = nc.dram_tensor("b1", [P, NFF], F32, kind="ExternalInput")
    b2_d = nc.dram_tensor("b2", [P, NCK], F32, kind="ExternalInput")
    out_d = nc.dram_tensor("outT", [P, NCK * MT], BF16, kind="ExternalOutput")

    wv_tiles = {}
    wm_tiles = {}
    ln2_ps = {}
    ln2_sc = {}
    ov_hold = {}

    with tile.TileContext(nc) as tc:
        with (
            tc.tile_pool(name="const", bufs=1) as cpool,
            tc.tile_pool(name="persist", bufs=1) as big,
            tc.tile_pool(name="sc", bufs=4) as sc,
            tc.tile_pool(name="sq", bufs=2) as sqp,
            tc.tile_pool(name="tmpb", bufs=2) as tmpp,
            tc.tile_pool(name="x2t", bufs=2) as x2tp,
            tc.tile_pool(name="ln2", bufs=4) as ln2p,
            tc.tile_pool(name="ln2s", bufs=2) as ln2sp,
            tc.tile_pool(name="wblk", bufs=16) as wblk,
            tc.tile_pool(name="w2s", bufs=2) as w2s,
            tc.tile_pool(name="pt", bufs=3) as ptp,
            tc.tile_pool(name="rc", bufs=2) as rcp,
            tc.tile_pool(name="outts", bufs=2) as outts,
            tc.tile_pool(name="psA", bufs=2, space="PSUM") as psA,
            tc.tile_pool(name="psF", bufs=1, space="PSUM") as psF,
            tc.tile_pool(name="psO", bufs=3, space="PSUM") as psO,
        ):
            # ---- constants / biases ----
            ones128 = cpool.tile([P, P], BF16, tag="ones128")
            nc.vector.memset(ones128[:], 1.0)
            ones64 = cpool.tile([1, HD], BF16, tag="ones64")
            nc.vector.memset(ones64[:], RECS * OESUB)
            eps_t = cpool.tile([P, 1], F32, tag="eps")
            nc.vector.memset(eps_t[:], EPS)
            nm2_t = cpool.tile([P, 1], F32, tag="nm2")
            nc.vector.memset(nm2_t[:], -2.0)
            dumv = cpool.tile([1, 8], F32, tag="dumv")
            bqk_t = cpool.tile([P, 16], F32, tag="bqk")
            bm_t = cpool.tile([P, NCK], F32, tag="bm")
            b1_t = cpool.tile([P, NFF], F32, tag="b1")
            b2_t = cpool.tile([P, NCK], F32, tag="b2")

            def tdve(ap):
                """Absorb a DMA's semaphore onto the DVE clock."""
                nc.vector.tensor_max(dumv[0:1, 0:1], ap, ap)

            def tpe(ap):
                """Absorb a weight-DMA's semaphore onto the PE clock."""
                nc.tensor.ldweights(ap)

            # ---- persistent activations ----
            xt = big.tile([P, NCK, NT], BF16, tag="xt")
            xnT = big.tile([P, NCK, NT], FP8, tag="xnT")
            kE = big.tile([P, 2, 4, NT], FP8, tag="kE")
            qE = big.tile([P, 2, 4, MT], FP8, tag="qE")
            vE = big.tile([P, 4, 2, H, HD + 1], FP8, tag="vE")
            oE = big.tile([P, NCK, MT], FP8, tag="oE")
            x2 = big.tile([P, NCK, MT], BF16, tag="x2")
            x2n = big.tile([P, NCK, MT], BF16, tag="x2n")
            h1T = big.tile([P, NFF, MT], BF16, tag="h1T")
            w1R = big.tile([P, NFF, NCK, P], BF16, tag="w1R")

            inv1 = big.tile([P, 2, 512], BF16, tag="inv1")
            ngm1 = big.tile([P, 2, 512], BF16, tag="ngm1")
            DBG.update(xnT=xnT, kE=kE, qE=qE, vE=vE, oE=oE, x2=x2,
                       x2n=x2n, h1T=h1T, inv1=inv1, ngm1=ngm1)

            nc.vector.memset(vE[:, :, :, :, HD:HD + 1], 1.0)

            # wblk ring slot plan (16 bufs): wk 0-7, wq 8-15, wv 0-7 (after
            # K chains), wm 8-15 (after Q chains) — no cross-stream cycles.
            # wk tiles allocated first (ring order); DMAs issued on Pool
            # after the xt stream so LN1 stats aren't delayed.
            wk_tiles = {}
            for c in range(NCK):
                wk_tiles[c] = wblk.tile([P, 4, 2, P], FP8, tag="wblk",
                                        name=f"wk{c}")

            # =============== LN1 stats (single x pass) ===============
            pssq = [psA.tile([P, 1024], F32, tag="ps", name=f"pssq{tb}")
                    for tb in range(2)]
            for k in range(NCK):
                if k % 2 == 0:
                    nc.sync.dma_start(xt[:, k, :], xt_d[:, ts(k, NT)])
                else:
                    nc.gpsimd.dma_start(xt[:, k, :], xt_d[:, ts(k, NT)])
                tdve(xt[0:1, k, 0:1])
                for tb in range(2):
                    sq = sqp.tile([P, 512], BF16, tag="sq",
                                  name=f"sqB{k}_{tb}")
                    with nc.allow_low_precision(reason="ln stats bf16"):
                        nc.scalar.square(sq[:], xt[:, k, ts(tb, 512)])
                    nc.tensor.matmul(pssq[tb][:, 0:512], ones128[:],
                                     xt[:, k, ts(tb, 512)],
                                     start=(k == 0), stop=(k == NCK - 1),
                                     skip_group_check=True)
                    nc.tensor.matmul(pssq[tb][:, 512:1024], ones128[:],
                                     sq[:],
                                     start=(k == 0), stop=(k == NCK - 1),
                                     skip_group_check=True)

            # weight/bias DMA issue, after xt so stats aren't stalled
            for c in range(NCK):
                nc.gpsimd.dma_start(wk_tiles[c][:], wk_d[c])
                tpe(wk_tiles[c][0:1, 0, 0, 0:1])
            nc.sync.dma_start(bqk_t[:], bqk_d[:])
            nc.sync.dma_start(bm_t[:], bm_d[:])
            nc.sync.dma_start(b1_t[:], b1_d[:])
            nc.sync.dma_start(b2_t[:], b2_d[:])

            for tb in range(2):
                mu = sc.tile([P, 512], BF16, tag="scb", name=f"mu1_{tb}")
                with nc.allow_low_precision(reason="ln stats bf16"):
                    nc.scalar.activation(mu[:], pssq[tb][:, 0:512], AF.Copy,
                                         scale=1.0 / C)
                musq = sc.tile([P, 512], BF16, tag="scb", name=f"musq1_{tb}")
                nc.vector.tensor_mul(musq[:], mu[:], mu[:])
                var = sc.tile([P, 512], BF16, tag="scb", name=f"var1_{tb}")
                with nc.allow_low_precision(reason="ln stats bf16"):
                    nc.vector.scalar_tensor_tensor(
                        var[:], pssq[tb][:, 512:1024], 1.0 / C, musq[:],
                        op0=OP.mult, op1=OP.subtract)
                std = sc.tile([P, 512], BF16, tag="scb", name=f"std1_{tb}")
                nc.scalar.activation(std[:], var[:], AF.Sqrt, bias=eps_t[:])
                with nc.allow_low_precision(reason="ln scale bf16"):
                    nc.vector.reciprocal(inv1[:, tb, :], std[:])
                    nc.vector.scalar_tensor_tensor(
                        ngm1[:, tb, :], mu[:], -1.0, inv1[:, tb, :],
                        op0=OP.mult, op1=OP.mult)

            # =============== LN1 apply (from SBUF) -> xnT fp8 ===============
            for k in range(NCK):
                for tb in range(2):
                    tmp = tmpp.tile([P, 512], BF16, tag="tmpb",
                                    name=f"lt{k}_{tb}")
                    nc.vector.tensor_mul(tmp[:], xt[:, k, ts(tb, 512)],
                                         inv1[:, tb, :])
                    with nc.allow_low_precision(reason="fp8 activations"):
                        nc.vector.tensor_add(xnT[:, k, ts(tb, 512)], tmp[:],
                                             ngm1[:, tb, :])

            # =============== Q / K projections (DoubleRow fp8) ===============
            # chunk c = lohi*4 + hg holds perm'd out-channels (see host prep)
            def qk_chain(ps_slice, w, qsl):
                for t in range(4):
                    nc.tensor.matmul(ps_slice, w[:, t, :, :],
                                     xnT[:, 2 * t:2 * t + 2, qsl],
                                     start=(t == 0), stop=(t == 3),
                                     perf_mode=DR, skip_group_check=True)

            for i in range(4):
                ps = psA.tile([P, 1024], F32, tag="ps", name=f"psq{i}")
                for half in range(2):
                    c = 2 * i + half
                    w = wblk.tile([P, 4, 2, P], FP8, tag="wblk",
                                  name=f"wq{c}")
                    nc.sync.dma_start(w[:], wq_d[c])
                    tpe(w[0:1, 0, 0, 0:1])
                    qk_chain(ps[:, ts(half, 512)], w, slice(0, MT))
                for half in range(2):
                    c = 2 * i + half
                    lohi, hg = c // 4, c % 4
                    with nc.allow_low_precision(reason="fp8 activations"):
                        nc.scalar.activation(
                            qE[:, lohi, hg, :], ps[:, ts(half, 512)],
                            AF.Identity, bias=bqk_t[:, c:c + 1])
            # wv on Pool (ring slots 0-7, reusing wk slots after K chains)
            for vb in range(2):
                for t in range(4):
                    w = wblk.tile([P, 2, 512], FP8, tag="wblk",
                                  name=f"wv{vb}_{t}")
                    nc.gpsimd.dma_start(w[:], wv_d[vb, t])
                    tpe(w[0:1, 0, 0:1])
                    wv_tiles[(vb, t)] = w
            # wm upfront on SP (slots 8-15 after wq), then w1 resident on SP
            for o in range(NCK):
                w = wblk.tile([P, 4, 2, P], FP8, tag="wblk", name=f"wm{o}")
                nc.sync.dma_start(w[:], wm_d[o])
                tpe(w[0:1, 0, 0, 0:1])
                wm_tiles[o] = w
            for f in range(NFF):
                nc.sync.dma_start(w1R[:, f, :, :], w1_d[f])
            tdve(w1R[0:1, 0, 0, 0:1])
            # K chains ordered so head-group hg's chunks (hg, hg+4) finish
            # first, letting window-0 scores start while K still runs
            for c in [0, 4, 1, 5, 2, 6, 3, 7]:
                ps = psA.tile([P, NT], F32, tag="ps", name=f"psk{c}")
                for tb in range(2):
                    qk_chain(ps[:, ts(tb, 512)], wk_tiles[c],
                             slice(tb * 512, tb * 512 + 512))
                lohi, hg = c // 4, c % 4
                with nc.allow_low_precision(reason="fp8 activations"):
                    nc.scalar.activation(kE[:, lohi, hg, :], ps[:],
                                         AF.Identity,
                                         bias=bqk_t[:, NCK + c:NCK + c + 1])

            # =============== V projection (DoubleRow fp8) ===============
            # out [128 tok, 512 vd] per (tok-chunk t8, vb); vE gets v_hat=32v
            pt_pre = {}

            def sc_exp(qb, h):
                b_, hg = h % 4, h // 4
                sp = psA.tile([P, NCK, QB], F32, tag="ps",
                              name=f"sp{qb}_{h}")
                for kc in range(NCK):
                    nc.tensor.matmul(
                        sp[:, kc, :],
                        kE[32 * b_:32 * b_ + 32, :, hg, ts(kc, P)],
                        qE[32 * b_:32 * b_ + 32, :, hg, ts(qb, QB)],
                        start=True, stop=True, perf_mode=DR,
                        skip_group_check=True,
                        tile_position=(32 * b_, 0))
                pt = ptp.tile([P, NCK, QB], FP8, tag="pt",
                              name=f"pt{qb}_{h}")
                with nc.allow_low_precision(reason="fp8 exp scores"):
                    nc.scalar.activation(pt[:], sp[:], AF.Exp,
                                         bias=nm2_t[:], scale=EXPS)
                return pt

            for t8 in range(NCK):
                ps = psA.tile([P, 1024], F32, tag="ps", name=f"psv{t8}")
                for vb in range(2):
                    for t in range(4):
                        nc.tensor.matmul(
                            ps[:, ts(vb, 512)],
                            xnT[:, 2 * t:2 * t + 2, ts(t8, P)],
                            wv_tiles[(vb, t)][:],
                            start=(t == 0), stop=(t == 3),
                            perf_mode=DR, skip_group_check=True)
                jg, pr = t8 // 2, t8 % 2
                for vb in range(2):
                    # spread the drain ops over DVE and ACT so neither
                    # serial queue gates the first PV (GPSIMD can't read
                    # PSUM per the BIR verifier)
                    with nc.allow_low_precision(reason="fp8 acts"):
                        if (2 * t8 + vb) % 2 == 0:
                            nc.vector.tensor_scalar_mul(
                                vE[:, jg, pr, ts(vb, 8), 0:HD],
                                ps[:, ts(vb, 512)].rearrange(
                                    "p (h d) -> p h d", d=HD), 1.0)
                        else:
                            nc.scalar.copy(
                                vE[:, jg, pr, ts(vb, 8), 0:HD],
                                ps[:, ts(vb, 512)].rearrange(
                                    "p (h d) -> p h d", d=HD))
                if t8 >= 6:
                    # scores/exp for window-0's first heads overlap the V
                    # tail (scores never read vE, so no PV deadlock)
                    pt_pre[t8 - 6] = sc_exp(0, t8 - 6)

            # =============== pipelined attention + MLP-front ===============
            def mlpa_thunks(qb):
                """proj+LN2+fc1 work units for query block qb (deps in
                order); emitted interleaved with attention of block qb+1."""
                th = []

                def proj_half(hf):
                    def f():
                        ps = psF.tile([P, 4, QB], F32, tag="pf",
                                      name=f"pm{qb}_{hf}")
                        for o in range(4 * hf, 4 * hf + 4):
                            wt = wm_tiles[o]
                            for t in range(4):
                                nc.tensor.matmul(
                                    ps[:, o - 4 * hf, :], wt[:, t, :, :],
                                    oE[:, 2 * t:2 * t + 2, ts(qb, QB)],
                                    start=(t == 0), stop=(t == 3),
                                    perf_mode=DR, skip_group_check=True)
                        for o in range(4 * hf, 4 * hf + 4):
                            t_ = x2tp.tile([P, QB], BF16, tag="x2t",
                                           name=f"x2t{qb}_{o}")
                            nc.vector.tensor_scalar(
                                t_[:], ps[:, o - 4 * hf, :], PROJS,
                                bm_t[:, o:o + 1], op0=OP.mult, op1=OP.add)
                            with nc.allow_low_precision(reason="x2 bf16"):
                                nc.vector.tensor_add(
                                    x2[:, o, ts(qb, QB)], t_[:],
                                    xt[:, o, qb * QB:qb * QB + QB])
                    return f
                th.append(proj_half(0))
                th.append(proj_half(1))

                def ln2_stats():
                    # sequential chains (sq first, then x): interleaved
                    # chains in one 2KB zero region corrupt each other via
                    # pending-zero re-marking; sequential chains are safe
                    psa = psO.tile([P, 4, QB], F32, tag="ov",
                                   name=f"pl2_{qb}")
                    ln2_ps[qb] = psa
                    for k in range(NCK):
                        sq2 = sqp.tile([P, QB], BF16, tag="sq",
                                       name=f"sq2_{qb}_{k}")
                        nc.gpsimd.tensor_mul(sq2[:], x2[:, k, ts(qb, QB)],
                                             x2[:, k, ts(qb, QB)])
                        nc.tensor.matmul(psa[:, 1, :], ones128[:], sq2[:],
                                         start=(k == 0), stop=(k == NCK - 1),
                                         skip_group_check=True)
                    for k in range(NCK):
                        nc.tensor.matmul(psa[:, 0, :], ones128[:],
                                         x2[:, k, ts(qb, QB)],
                                         start=(k == 0), stop=(k == NCK - 1),
                                         skip_group_check=True)
                th.append(ln2_stats)

                def ln2_fin():
                    psa = ln2_ps.pop(qb)
                    psb = psa[:, 1:2, :]
                    mu = ln2p.tile([P, QB], F32, tag="l2", name=f"mu2_{qb}")
                    nc.scalar.activation(mu[:], psa[:, 0, :], AF.Copy,
                                         scale=1.0 / C)
                    musq = ln2p.tile([P, QB], F32, tag="l2",
                                     name=f"msq2_{qb}")
                    nc.vector.tensor_mul(musq[:], mu[:], mu[:])
                    var = ln2p.tile([P, QB], F32, tag="l2", name=f"var2_{qb}")
                    nc.vector.scalar_tensor_tensor(
                        var[:], psb[:, 0, :], 1.0 / C, musq[:],
                        op0=OP.mult, op1=OP.subtract)
                    # inv-std = exp(-0.5*ln(var+eps)): Ln and Exp share an
                    # ACT table, so no table switch amid the exp stream
                    lv = ln2p.tile([P, QB], F32, tag="l2", name=f"lv2_{qb}")
                    nc.scalar.activation(lv[:], var[:], AF.Ln, bias=eps_t[:])
                    iv = ln2sp.tile([P, QB], BF16, tag="iv2",
                                    name=f"iv2_{qb}")
                    ng = ln2sp.tile([P, QB], BF16, tag="ng2",
                                    name=f"ng2_{qb}")
                    with nc.allow_low_precision(reason="ln scale bf16"):
                        nc.scalar.activation(iv[:], lv[:], AF.Exp, scale=-0.5)
                        nc.vector.scalar_tensor_tensor(
                            ng[:], mu[:], -1.0, iv[:],
                            op0=OP.mult, op1=OP.mult)
                    ln2_sc[qb] = (iv, ng)
                    DBG[f"iv2_{qb}"] = iv
                    DBG[f"ng2_{qb}"] = ng
                    DBG[f"mu2_{qb}"] = mu
                    DBG[f"var2_{qb}"] = var
                th.append(ln2_fin)

                def x2n_w(half):
                    def f():
                        iv, ng = ln2_sc[qb]
                        for k in range(4 * half, 4 * half + 4):
                            t_ = x2tp.tile([P, QB], BF16, tag="x2t",
                                           name=f"xnt{qb}_{k}")
                            nc.gpsimd.tensor_mul(t_[:], x2[:, k, ts(qb, QB)],
                                                 iv[:])
                            with nc.allow_low_precision(reason="x2n bf16"):
                                nc.gpsimd.tensor_add(x2n[:, k, ts(qb, QB)],
                                                     t_[:], ng[:])
                    return f
                th.append(x2n_w(0))
                th.append(x2n_w(1))

                def fc1_grp(g):
                    def f():
                        ps = psF.tile([P, 4, QB], F32, tag="pf",
                                      name=f"p1_{qb}_{g}")
                        for fi in range(4):
                            fch = 4 * g + fi
                            for k in range(NCK):
                                nc.tensor.matmul(
                                    ps[:, fi, :], w1R[:, fch, k, :],
                                    x2n[:, k, ts(qb, QB)],
                                    start=(k == 0), stop=(k == NCK - 1),
                                    skip_group_check=True)
                        # drain psum -> h1T (gelu deferred to tail);
                        # GPSIMD can't read PSUM: alternate DVE/ACT, but
                        # keep qb3's drains off ACT so the chased gelus
                        # don't serialize the single psF slot
                        if qb == NQB - 1 or g % 2 == 0:
                            nc.vector.tensor_scalar_mul(
                                h1T[:, 4 * g:4 * g + 4, ts(qb, QB)],
                                ps[:], 1.0)
                        else:
                            nc.scalar.copy(
                                h1T[:, 4 * g:4 * g + 4, ts(qb, QB)], ps[:])
                        if qb == NQB - 1:
                            # last block: gelu chases fc1 so fc2 can stream
                            for fi in range(4):
                                fch = 4 * g + fi
                                nc.scalar.activation(
                                    h1T[:, fch, :], h1T[:, fch, :], AF.Gelu,
                                    bias=b1_t[:, fch:fch + 1])
                    return f
                f1 = [fc1_grp(g) for g in range(NFF // 4)]
                return th, f1

            pend = []
            fc1s = {}
            for qb in range(NQB):
                for h in range(H):
                    e = h % 2
                    if qb == 0 and h in pt_pre:
                        pt = pt_pre.pop(h)
                    else:
                        pt = sc_exp(qb, h)
                    ov = psO.tile([HD + 1, QB], F32, tag="ov",
                                  name=f"ov{qb}_{h}")
                    for a in range(4):
                        nc.tensor.matmul(ov[:], vE[:, a, :, h, :],
                                         pt[:, 2 * a:2 * a + 2, :],
                                         start=(a == 0), stop=(a == 3),
                                         perf_mode=DR, skip_group_check=True)
                    rc = rcp.tile([1, QB], BF16, tag="rc",
                                  name=f"rc{qb}_{h}")
                    with nc.allow_low_precision(reason="softmax denom bf16"):
                        nc.vector.reciprocal(rc[:], ov[64:65, :])
                    # stage ov into oE (SBUF) first: walrus allows only one
                    # PSUM input per DVE op, so the bc multiply is in-place
                    ch = h // 2
                    with nc.allow_low_precision(reason="fp8 oE"):
                        if e == 0:
                            nc.vector.tensor_scalar_mul(
                                oE[0:HD, ch, ts(qb, QB)], ov[0:HD, :],
                                1.0 / OESUB)
                        else:
                            nc.scalar.mul(oE[HD:P, ch, ts(qb, QB)],
                                          ov[0:HD, :], 1.0 / OESUB)
                    if e == 0:
                        ov_hold[0] = rc
                    else:
                        rc0 = ov_hold.pop(0)
                        bcp = psO.tile([P, QB], F32, tag="ov",
                                       name=f"bc{qb}_{ch}")
                        nc.tensor.matmul(bcp[0:HD, :], ones64[:], rc0[:],
                                         start=True, stop=True,
                                         skip_group_check=True)
                        nc.tensor.matmul(bcp[HD:P, :], ones64[:], rc[:],
                                         start=True, stop=True,
                                         skip_group_check=True)
                        with nc.allow_low_precision(reason="fp8 oE"):
                            nc.vector.tensor_mul(
                                oE[0:HD, ch, ts(qb, QB)],
                                oE[0:HD, ch, ts(qb, QB)], bcp[0:HD, :])
                            nc.vector.tensor_mul(
                                oE[HD:P, ch, ts(qb, QB)],
                                oE[HD:P, ch, ts(qb, QB)], bcp[HD:P, :])
                    # interleave one pending MLP unit per head slot
                    if pend:
                        pend.pop(0)()
                for t_ in pend:
                    t_()
                fr, f1 = mlpa_thunks(qb)
                # window qb+1 runs front(qb) plus fc1(qb-1): the serial
                # proj->LN2->x2n chain gets a full window of attention
                # cover before its fc1 consumes it one window later
                pend = fr + fc1s.get(qb - 1, [])
                fc1s[qb] = f1
            for t_ in pend:
                t_()
            for t_ in fc1s[NQB - 1]:
                t_()

            # =============== tail: fc2 (gelu already chased fc1) ===============
            for i in range(4):
                ps = psA.tile([P, 1024], F32, tag="ps", name=f"psf2_{i}")
                for half in range(2):
                    o = 2 * i + half
                    for fh in range(2):
                        w2t = w2s.tile([P, NFF // 2, P], BF16, tag="w2f",
                                       name=f"w2_{o}_{fh}")
                        nc.sync.dma_start(
                            w2t[:], w2_d[o][:, fh * 2048:(fh + 1) * 2048])
                        tpe(w2t[0:1, 0, 0:1])
                        for fi in range(NFF // 2):
                            f = fh * (NFF // 2) + fi
                            nc.tensor.matmul(
                                ps[:, ts(half, 512)], w2t[:, fi, :],
                                h1T[:, f, :],
                                start=(f == 0), stop=(f == NFF - 1),
                                skip_group_check=True)
                for half in range(2):
                    o = 2 * i + half
                    outt = outts.tile([P, MT], BF16, tag="outt",
                                      name=f"out{o}")
                    with nc.allow_low_precision(reason="bf16 output"):
                        nc.vector.scalar_tensor_tensor(
                            outt[:], ps[:, ts(half, 512)], b2_t[:, o:o + 1],
                            x2[:, o, :], op0=OP.add, op1=OP.add)
                    if o % 2 == 0:
                        nc.sync.dma_start(out_d[:, ts(o, 512)], outt[:])
                    else:
                        nc.gpsimd.dma_start(out_d[:, ts(o, 512)], outt[:])

    nc.compile()
    return nc


# ---------------- host side ----------------

def _bf16(a):
    return np.ascontiguousarray(a.astype(ml_dtypes.bfloat16))


def _f32(a):
    return np.ascontiguousarray(a.astype(np.float32))


def _fp8(a):
    return np.ascontiguousarray(
        np.clip(a, -240.0, 240.0).astype(ml_dtypes.float8_e4m3))


def _qk_perm():
    """out-channel permutation: chunk c = lohi*4+hg, partition p = b*32+r
    holds orig channel 64*(4*hg+b) + 32*lohi + r."""
    perm = np.empty(C, np.int64)
    for c in range(NCK):
        lohi, hg = c // 4, c % 4
        for p in range(P):
            b_, r = p // 32, p % 32
            perm[c * P + p] = 64 * (4 * hg + b_) + 32 * lohi + r
    return perm


def _dr_pack(W):
    """[out (nck*128), in C] -> [nck, P, (t, j, m)] DoubleRow layout:
    element [c][p][t, j, m] = W[c*128+m, (2t+j)*128+p]."""
    nck = W.shape[0] // P
    Wr = W.reshape(nck, P, NCK, P)          # [c, m, kin, p]
    out = np.empty((nck, P, 4, 2, P), W.dtype)
    for t in range(4):
        for j in range(2):
            out[:, :, t, j, :] = Wr[:, :, 2 * t + j, :].transpose(0, 2, 1)
    return out.reshape(nck, P, C)


def prepare_inputs(x, qkv_w, qkv_b, attn_proj_w, attn_proj_b, blk_proj_w,
                   blk_proj_b, ln1_g, ln1_b, ln2_g, ln2_b, fc1_w, fc1_b,
                   fc2_w, fc2_b, mask):
    x = np.asarray(x, np.float32)
    qkv_w = np.asarray(qkv_w, np.float64)
    qkv_b = np.asarray(qkv_b, np.float64)

    g1 = np.asarray(ln1_g, np.float64)
    bl1 = np.asarray(ln1_b, np.float64)
    Wq = qkv_w[0:C] * g1[None, :]
    bq = qkv_w[0:C] @ bl1 + qkv_b[0:C]
    Wk = qkv_w[C:2 * C] * g1[None, :]
    bk = qkv_w[C:2 * C] @ bl1 + qkv_b[C:2 * C]
    Wv = qkv_w[2 * C:] * g1[None, :]
    bv = qkv_w[2 * C:] @ bl1 + qkv_b[2 * C:]

    A = np.asarray(attn_proj_w, np.float64)
    Bw = np.asarray(blk_proj_w, np.float64)
    Wm = Bw @ A
    bm = Wm @ bv + Bw @ np.asarray(attn_proj_b, np.float64) \
        + np.asarray(blk_proj_b, np.float64)

    g2 = np.asarray(ln2_g, np.float64)
    bl2 = np.asarray(ln2_b, np.float64)
    W1 = np.asarray(fc1_w, np.float64) * g2[None, :]
    b1 = np.asarray(fc1_w, np.float64) @ bl2 + np.asarray(fc1_b, np.float64)
    W2 = np.asarray(fc2_w, np.float64)
    b2 = np.asarray(fc2_b, np.float64)

    perm = _qk_perm()
    wq_l = _fp8(_dr_pack((SW * Wq)[perm]))
    wk_l = _fp8(_dr_pack((SW * Wk)[perm]))
    bqP = (SW * bq)[perm]
    bkP = (SW * bk)[perm]
    # V: [vb][t][p][(j, n)]: SW * Wv[vb*512+n, (2t+j)*128+p]
    WvS = (SW * Wv).reshape(2, 512, NCK, P)     # [vb, n, kin, p]
    wv_l = np.empty((2, 4, P, 2, 512), np.float64)
    for t in range(4):
        for j in range(2):
            wv_l[:, t, :, j, :] = WvS[:, :, 2 * t + j, :].transpose(0, 2, 1)
    wv_l = _fp8(wv_l.reshape(2, 4, P, 1024))
    wm_l = _fp8(_dr_pack(SW * Wm))
    w1_l = _bf16(W1.reshape(NFF, P, NCK, P).transpose(0, 3, 2, 1)
                 .reshape(NFF, P, C))
    w2_l = _bf16(W2.reshape(NCK, P, NFF, P).transpose(0, 3, 2, 1)
                 .reshape(NCK, P, FF))
    bqk_l = _f32(np.concatenate([bqP.reshape(NCK, P).T,
                                 bkP.reshape(NCK, P).T], axis=1))
    bm_l = _f32(bm.reshape(NCK, P).T)
    b1_l = _f32(b1.reshape(NFF, P).T)
    b2_l = _f32(b2.reshape(NCK, P).T)

    shared = dict(wq=wq_l, wk=wk_l, wv=wv_l, wm=wm_l, w1=w1_l, w2=w2_l,
                  bqk=bqk_l, bm=bm_l, b1=b1_l, b2=b2_l)

    in_maps = []
    for core in range(8):
        b_, m = divmod(core, 2)
        xb = x[b_]
        xp = np.concatenate([xb[m * MT:(m + 1) * MT],
                             xb[(1 - m) * MT:(2 - m) * MT]], axis=0)
        xt_l = _bf16(xp.reshape(NT, NCK, P).transpose(2, 1, 0)
                     .reshape(P, NCK * NT))
        in_maps.append(dict(shared, xt=xt_l))
    return in_maps


def gather_output(results):
    out = np.empty((B, N, C), np.float32)
    for core in range(8):
        b_, m = divmod(core, 2)
        O = np.asarray(results[core]["outT"]).astype(np.float32)
        O = O.reshape(P, NCK, MT)
        out[b_, m * MT:(m + 1) * MT, :] = O.transpose(2, 1, 0).reshape(MT, C)
    return out


_CACHE = {}


def kernel(**inputs):
    if "nc" not in _CACHE:
        _CACHE["nc"] = build_module()
    nc = _CACHE["nc"]
    in_maps = prepare_inputs(**inputs)
    res = run_bass_kernel_spmd(nc, in_maps, core_ids=list(range(8)))
    return gather_output(res.results)


# revision 14
# speedup vs baseline: 1.2815x; 1.0224x over previous
"""Trainium2 Bass kernel for nn_Block_44358422233377 (dense transformer block).

v2: fp8e4m3+DoubleRow attention side (4x cheaper per MAC in the cost model),
bf16 MLP, 4-deep query-block software pipeline overlapping the ACT-bound
softmax-exp with PE-bound MLP-front work, single-pass x streaming, DMA issue
split across SP (x, attn weights, w2, out) and Pool (w1, fc1-psum drains).

Sharding: core c = (batch b = c//2, query-half m = c%2); K/V recomputed per
sibling (no collectives). Activations live transposed [channels(part), tok].

Numerics: attn weights *32 -> e4m3; scores psum = 1024*s_true; softmax via
exp(s_raw/8192 - 2) in e4m3 (denominator via the ones-column of V; the
common shift cancels in the normalize). oE staged as ov/512 in fp8, then
scaled in place by the PE-broadcast 2048*RECS'/denom; merged-proj descale
1/(SW*SW*RECS) folded into the x2 write. MLP stays bf16 (fp8 fails the
2e-2 gate). Output DMA'd bf16, upcast on host. End-to-end rel err 7.4e-3.

Walrus BIR rules honored (the CoreSim-only version broke all three):
GPSIMD never touches PSUM; DVE/ACT ops read at most one PSUM operand;
two SBUF inputs of a DVE op share a partition base. Interleaved matmul
accumulation chains never share a 2KB PSUM zero region (pending-zero
re-marking silently zeroes the neighbor chain's partials).
"""
import sys

sys.path.insert(0, "/opt/trn_rl_repo")

import numpy as np
import ml_dtypes

import concourse.bass as bass
import concourse.bacc as bacc
import concourse.mybir as mybir
import concourse.tile as tile
from concourse.bass import ts
from concourse.bass_utils import run_bass_kernel_spmd

F32 = mybir.dt.float32
BF16 = mybir.dt.bfloat16
FP8 = mybir.dt.float8e4
AF = mybir.ActivationFunctionType
OP = mybir.AluOpType
DR = mybir.MatmulPerfMode.DoubleRow

P = 128
B, N, C, H = 4, 1024, 1024, 16
HD = C // H          # 64
FF = 4 * C           # 4096
NT = N               # context tokens per core
MT = N // 2          # own (query) tokens per core
QB = 128             # query sub-block (pipeline granularity)
NQB = MT // QB       # 4
EPS = 1e-6
NCK = C // P         # 8 channel chunks
NFF = FF // P        # 32 ff chunks
SW = 32.0            # fp8 weight scale
EXPS = 1.0 / (8.0 * SW * SW)       # exp scale  (= 1/8192)
RECS = 4.0                         # oE scale (vs o_true: SW*RECS)
OESUB = 512.0                      # staging scale: oE_pre = ov/OESUB
PROJS = 1.0 / (SW * SW * RECS)     # proj psum descale (= 1/16384)


DBG = {}


def build_module():
    nc = bacc.Bacc("TRN2", target_bir_lowering=False, debug=False)

    xt_d = nc.dram_tensor("xt", [P, NCK * NT], BF16, kind="ExternalInput")
    wq_d = nc.dram_tensor("wq", [NCK, P, C], FP8, kind="ExternalInput")
    wk_d = nc.dram_tensor("wk", [NCK, P, C], FP8, kind="ExternalInput")
    wv_d = nc.dram_tensor("wv", [2, 4, P, 1024], FP8, kind="ExternalInput")
    wm_d = nc.dram_tensor("wm", [NCK, P, C], FP8, kind="ExternalInput")
    w1_d = nc.dram_tensor("w1", [NFF, P, C], BF16, kind="ExternalInput")
    w2_d = nc.dram_tensor("w2", [NCK, P, FF], BF16, kind="ExternalInput")
    bqk_d = nc.dram_tensor("bqk", [P, 16], F32, kind="ExternalInput")
    bm_d = nc.dram_tensor("bm", [P, NCK], F32, kind="ExternalInput")
    b1_d = nc.dram_tensor("b1", [P, NFF], F32, kind="ExternalInput")
    b2_d = nc.dram_tensor("b2", [P, NCK], F32, kind="ExternalInput")
    out_d = nc.dram_tensor("outT", [P, NCK * MT], BF16, kind="ExternalOutput")

    wv_tiles = {}
    wm_tiles = {}
    ln2_ps = {}
    ln2_sc = {}
    ov_hold = {}

    with tile.TileContext(nc) as tc:
        with (
            tc.tile_pool(name="const", bufs=1) as cpool,
            tc.tile_pool(name="persist", bufs=1) as big,
            tc.tile_pool(name="sc", bufs=4) as sc,
            tc.tile_pool(name="sq", bufs=2) as sqp,
            tc.tile_pool(name="tmpb", bufs=2) as tmpp,
            tc.tile_pool(name="x2t", bufs=2) as x2tp,
            tc.tile_pool(name="ln2", bufs=4) as ln2p,
            tc.tile_pool(name="ln2s", bufs=2) as ln2sp,
            tc.tile_pool(name="wblk", bufs=16) as wblk,
            tc.tile_pool(name="w2s", bufs=2) as w2s,
            tc.tile_pool(name="pt", bufs=3) as ptp,
            tc.tile_pool(name="rc", bufs=2) as rcp,
            tc.tile_pool(name="outts", bufs=2) as outts,
            tc.tile_pool(name="psA", bufs=2, space="PSUM") as psA,
            tc.tile_pool(name="psF", bufs=1, space="PSUM") as psF,
            tc.tile_pool(name="psO", bufs=3, space="PSUM") as psO,
        ):
            # ---- constants / biases ----
            ones128 = cpool.tile([P, P], BF16, tag="ones128")
            nc.vector.memset(ones128[:], 1.0)
            ones64 = cpool.tile([1, HD], BF16, tag="ones64")
            nc.vector.memset(ones64[:], RECS * OESUB)
            eps_t = cpool.tile([P, 1], F32, tag="eps")
            nc.vector.memset(eps_t[:], EPS)
            nm2_t = cpool.tile([P, 1], F32, tag="nm2")
            nc.vector.memset(nm2_t[:], -2.0)
            dumv = cpool.tile([1, 8], F32, tag="dumv")
            bqk_t = cpool.tile([P, 16], F32, tag="bqk")
            bm_t = cpool.tile([P, NCK], F32, tag="bm")
            b1_t = cpool.tile([P, NFF], F32, tag="b1")
            b2_t = cpool.tile([P, NCK], F32, tag="b2")

            def tdve(ap):
                """Absorb a DMA's semaphore onto the DVE clock."""
                nc.vector.tensor_max(dumv[0:1, 0:1], ap, ap)

            def tpe(ap):
                """Absorb a weight-DMA's semaphore onto the PE clock."""
                nc.tensor.ldweights(ap)

            # ---- persistent activations ----
            xt = big.tile([P, NCK, NT], BF16, tag="xt")
            xnT = big.tile([P, NCK, NT], FP8, tag="xnT")
            kE = big.tile([P, 2, 4, NT], FP8, tag="kE")
            qE = big.tile([P, 2, 4, MT], FP8, tag="qE")
            vE = big.tile([P, 4, 2, H, HD + 1], FP8, tag="vE")
            oE = big.tile([P, NCK, MT], FP8, tag="oE")
            x2 = big.tile([P, NCK, MT], BF16, tag="x2")
            x2n = big.tile([P, NCK, MT], BF16, tag="x2n")
            h1T = big.tile([P, NFF, MT], BF16, tag="h1T")
            w1R = big.tile([P, NFF, NCK, P], BF16, tag="w1R")

            inv1 = big.tile([P, 2, 512], BF16, tag="inv1")
            ngm1 = big.tile([P, 2, 512], BF16, tag="ngm1")
            DBG.update(xnT=xnT, kE=kE, qE=qE, vE=vE, oE=oE, x2=x2,
                       x2n=x2n, h1T=h1T, inv1=inv1, ngm1=ngm1)

            nc.vector.memset(vE[:, :, :, :, HD:HD + 1], 1.0)

            # wblk ring slot plan (16 bufs): wk 0-7, wq 8-15, wv 0-7 (after
            # K chains), wm 8-15 (after Q chains) — no cross-stream cycles.
            # wk tiles allocated first (ring order); DMAs issued on Pool
            # after the xt stream so LN1 stats aren't delayed.
            wk_tiles = {}
            for c in range(NCK):
                wk_tiles[c] = wblk.tile([P, 4, 2, P], FP8, tag="wblk",
                                        name=f"wk{c}")

            # =============== LN1 stats (single x pass) ===============
            pssq = [psA.tile([P, 1024], F32, tag="ps", name=f"pssq{tb}")
                    for tb in range(2)]
            for k in range(NCK):
                if k % 2 == 0:
                    nc.sync.dma_start(xt[:, k, :], xt_d[:, ts(k, NT)])
                else:
                    nc.gpsimd.dma_start(xt[:, k, :], xt_d[:, ts(k, NT)])
                tdve(xt[0:1, k, 0:1])
                for tb in range(2):
                    sq = sqp.tile([P, 512], BF16, tag="sq",
                                  name=f"sqB{k}_{tb}")
                    with nc.allow_low_precision(reason="ln stats bf16"):
                        nc.scalar.square(sq[:], xt[:, k, ts(tb, 512)])
                    nc.tensor.matmul(pssq[tb][:, 0:512], ones128[:],
                                     xt[:, k, ts(tb, 512)],
                                     start=(k == 0), stop=(k == NCK - 1),
                                     skip_group_check=True)
                    nc.tensor.matmul(pssq[tb][:, 512:1024], ones128[:],
                                     sq[:],
                                     start=(k == 0), stop=(k == NCK - 1),
                                     skip_group_check=True)

            # weight/bias DMA issue, after xt so stats aren't stalled
            for c in range(NCK):
                nc.gpsimd.dma_start(wk_tiles[c][:], wk_d[c])
                tpe(wk_tiles[c][0:1, 0, 0, 0:1])
            nc.sync.dma_start(bqk_t[:], bqk_d[:])
            nc.sync.dma_start(bm_t[:], bm_d[:])
            nc.sync.dma_start(b1_t[:], b1_d[:])
            nc.sync.dma_start(b2_t[:], b2_d[:])

            for tb in range(2):
                mu = sc.tile([P, 512], BF16, tag="scb", name=f"mu1_{tb}")
                with nc.allow_low_precision(reason="ln stats bf16"):
                    nc.scalar.activation(mu[:], pssq[tb][:, 0:512], AF.Copy,
                                         scale=1.0 / C)
                musq = sc.tile([P, 512], BF16, tag="scb", name=f"musq1_{tb}")
                nc.vector.tensor_mul(musq[:], mu[:], mu[:])
                var = sc.tile([P, 512], BF16, tag="scb", name=f"var1_{tb}")
                with nc.allow_low_precision(reason="ln stats bf16"):
                    nc.vector.scalar_tensor_tensor(
                        var[:], pssq[tb][:, 512:1024], 1.0 / C, musq[:],
                        op0=OP.mult, op1=OP.subtract)
                std = sc.tile([P, 512], BF16, tag="scb", name=f"std1_{tb}")
                nc.scalar.activation(std[:], var[:], AF.Sqrt, bias=eps_t[:])
                with nc.allow_low_precision(reason="ln scale bf16"):
                    nc.vector.reciprocal(inv1[:, tb, :], std[:])
                    nc.vector.scalar_tensor_tensor(
                        ngm1[:, tb, :], mu[:], -1.0, inv1[:, tb, :],
                        op0=OP.mult, op1=OP.mult)

            # =============== LN1 apply (from SBUF) -> xnT fp8 ===============
            for k in range(NCK):
                for tb in range(2):
                    tmp = tmpp.tile([P, 512], BF16, tag="tmpb",
                                    name=f"lt{k}_{tb}")
                    nc.vector.tensor_mul(tmp[:], xt[:, k, ts(tb, 512)],
                                         inv1[:, tb, :])
                    with nc.allow_low_precision(reason="fp8 activations"):
                        nc.vector.tensor_add(xnT[:, k, ts(tb, 512)], tmp[:],
                                             ngm1[:, tb, :])

            # =============== Q / K projections (DoubleRow fp8) ===============
            # chunk c = lohi*4 + hg holds perm'd out-channels (see host prep)
            def qk_chain(ps_slice, w, qsl):
                for t in range(4):
                    nc.tensor.matmul(ps_slice, w[:, t, :, :],
                                     xnT[:, 2 * t:2 * t + 2, qsl],
                                     start=(t == 0), stop=(t == 3),
                                     perf_mode=DR, skip_group_check=True)

            for i in range(4):
                ps = psA.tile([P, 1024], F32, tag="ps", name=f"psq{i}")
                for half in range(2):
                    c = 2 * i + half
                    w = wblk.tile([P, 4, 2, P], FP8, tag="wblk",
                                  name=f"wq{c}")
                    nc.sync.dma_start(w[:], wq_d[c])
                    tpe(w[0:1, 0, 0, 0:1])
                    qk_chain(ps[:, ts(half, 512)], w, slice(0, MT))
                for half in range(2):
                    c = 2 * i + half
                    lohi, hg = c // 4, c % 4
                    with nc.allow_low_precision(reason="fp8 activations"):
                        nc.scalar.activation(
                            qE[:, lohi, hg, :], ps[:, ts(half, 512)],
                            AF.Identity, bias=bqk_t[:, c:c + 1])
            # wv on Pool (ring slots 0-7, reusing wk slots after K chains)
            for vb in range(2):
                for t in range(4):
                    w = wblk.tile([P, 2, 512], FP8, tag="wblk",
                                  name=f"wv{vb}_{t}")
                    nc.gpsimd.dma_start(w[:], wv_d[vb, t])
                    tpe(w[0:1, 0, 0:1])
                    wv_tiles[(vb, t)] = w
            # wm upfront on SP (slots 8-15 after wq), then w1 resident on SP
            for o in range(NCK):
                w = wblk.tile([P, 4, 2, P], FP8, tag="wblk", name=f"wm{o}")
                nc.sync.dma_start(w[:], wm_d[o])
                tpe(w[0:1, 0, 0, 0:1])
                wm_tiles[o] = w
            for f in range(NFF):
                nc.sync.dma_start(w1R[:, f, :, :], w1_d[f])
            tdve(w1R[0:1, 0, 0, 0:1])
            # K chains ordered so head-group hg's chunks (hg, hg+4) finish
            # first, letting window-0 scores start while K still runs
            for c in [0, 4, 1, 5, 2, 6, 3, 7]:
                ps = psA.tile([P, NT], F32, tag="ps", name=f"psk{c}")
                for tb in range(2):
                    qk_chain(ps[:, ts(tb, 512)], wk_tiles[c],
                             slice(tb * 512, tb * 512 + 512))
                lohi, hg = c // 4, c % 4
                with nc.allow_low_precision(reason="fp8 activations"):
                    nc.scalar.activation(kE[:, lohi, hg, :], ps[:],
                                         AF.Identity,
                                         bias=bqk_t[:, NCK + c:NCK + c + 1])

            # =============== V projection (DoubleRow fp8) ===============
            # out [128 tok, 512 vd] per (tok-chunk t8, vb); vE gets v_hat=32v
            pt_pre = {}

            def sc_exp(qb, h):
                b_, hg = h % 4, h // 4
                sp = psA.tile([P, NCK, QB], F32, tag="ps",
                              name=f"sp{qb}_{h}")
                for kc in range(NCK):
                    nc.tensor.matmul(
                        sp[:, kc, :],
                        kE[32 * b_:32 * b_ + 32, :, hg, ts(kc, P)],
                        qE[32 * b_:32 * b_ + 32, :, hg, ts(qb, QB)],
                        start=True, stop=True, perf_mode=DR,
                        skip_group_check=True,
                        tile_position=(32 * b_, 0))
                pt = ptp.tile([P, NCK, QB], FP8, tag="pt",
                              name=f"pt{qb}_{h}")
                with nc.allow_low_precision(reason="fp8 exp scores"):
                    nc.scalar.activation(pt[:], sp[:], AF.Exp,
                                         bias=nm2_t[:], scale=EXPS)
                return pt

            for t8 in range(NCK):
                ps = psA.tile([P, 1024], F32, tag="ps", name=f"psv{t8}")
                for vb in range(2):
                    for t in range(4):
                        nc.tensor.matmul(
                            ps[:, ts(vb, 512)],
                            xnT[:, 2 * t:2 * t + 2, ts(t8, P)],
                            wv_tiles[(vb, t)][:],
                            start=(t == 0), stop=(t == 3),
                            perf_mode=DR, skip_group_check=True)
                jg, pr = t8 // 2, t8 % 2
                for vb in range(2):
                    # spread the drain ops over DVE and ACT so neither
                    # serial queue gates the first PV (GPSIMD can't read
                    # PSUM per the BIR verifier)
                    with nc.allow_low_precision(reason="fp8 acts"):
                        if (2 * t8 + vb) % 2 == 0:
                            nc.vector.tensor_scalar_mul(
                                vE[:, jg, pr, ts(vb, 8), 0:HD],
                                ps[:, ts(vb, 512)].rearrange(
                                    "p (h d) -> p h d", d=HD), 1.0)
                        else:
                            nc.scalar.copy(
                                vE[:, jg, pr, ts(vb, 8), 0:HD],
                                ps[:, ts(vb, 512)].rearrange(
                                    "p (h d) -> p h d", d=HD))
                if t8 >= 6:
                    # scores/exp for window-0's first heads overlap the V
                    # tail (scores never read vE, so no PV deadlock)
                    pt_pre[t8 - 6] = sc_exp(0, t8 - 6)

            # =============== pipelined attention + MLP-front ===============
            def mlpa_thunks(qb):
                """proj+LN2+fc1 work units for query block qb (deps in
                order); emitted interleaved with attention of block qb+1."""
                th = []

                def proj_half(hf):
                    def f():
                        ps = psF.tile([P, 4, QB], F32, tag="pf",
                                      name=f"pm{qb}_{hf}")
                        for o in range(4 * hf, 4 * hf + 4):
                            wt = wm_tiles[o]
                            for t in range(4):
                                nc.tensor.matmul(
                                    ps[:, o - 4 * hf, :], wt[:, t, :, :],
                                    oE[:, 2 * t:2 * t + 2, ts(qb, QB)],
                                    start=(t == 0), stop=(t == 3),
                                    perf_mode=DR, skip_group_check=True)
                        for o in range(4 * hf, 4 * hf + 4):
                            t_ = x2tp.tile([P, QB], BF16, tag="x2t",
                                           name=f"x2t{qb}_{o}")
                            nc.vector.tensor_scalar(
                                t_[:], ps[:, o - 4 * hf, :], PROJS,
                                bm_t[:, o:o + 1], op0=OP.mult, op1=OP.add)
                            with nc.allow_low_precision(reason="x2 bf16"):
                                nc.vector.tensor_add(
                                    x2[:, o, ts(qb, QB)], t_[:],
                                    xt[:, o, qb * QB:qb * QB + QB])
                    return f
                th.append(proj_half(0))
                th.append(proj_half(1))

                def ln2_stats():
                    # sequential chains (sq first, then x): interleaved
                    # chains in one 2KB zero region corrupt each other via
                    # pending-zero re-marking; sequential chains are safe
                    psa = psO.tile([P, 4, QB], F32, tag="ov",
                                   name=f"pl2_{qb}")
                    ln2_ps[qb] = psa
                    for k in range(NCK):
                        sq2 = sqp.tile([P, QB], BF16, tag="sq",
                                       name=f"sq2_{qb}_{k}")
                        nc.gpsimd.tensor_mul(sq2[:], x2[:, k, ts(qb, QB)],
                                             x2[:, k, ts(qb, QB)])
                        nc.tensor.matmul(psa[:, 1, :], ones128[:], sq2[:],
                                         start=(k == 0), stop=(k == NCK - 1),
                                         skip_group_check=True)
                    for k in range(NCK):
                        nc.tensor.matmul(psa[:, 0, :], ones128[:],
                                         x2[:, k, ts(qb, QB)],
                                         start=(k == 0), stop=(k == NCK - 1),
                                         skip_group_check=True)
                th.append(ln2_stats)

                def ln2_fin():
                    psa = ln2_ps.pop(qb)
                    psb = psa[:, 1:2, :]
                    mu = ln2p.tile([P, QB], F32, tag="l2", name=f"mu2_{qb}")
                    nc.scalar.activation(mu[:], psa[:, 0, :], AF.Copy,
                                         scale=1.0 / C)
                    musq = ln2p.tile([P, QB], F32, tag="l2",
                                     name=f"msq2_{qb}")
                    nc.vector.tensor_mul(musq[:], mu[:], mu[:])
                    var = ln2p.tile([P, QB], F32, tag="l2", name=f"var2_{qb}")
                    nc.vector.scalar_tensor_tensor(
                        var[:], psb[:, 0, :], 1.0 / C, musq[:],
                        op0=OP.mult, op1=OP.subtract)
                    # inv-std = exp(-0.5*ln(var+eps)): Ln and Exp share an
                    # ACT table, so no table switch amid the exp stream
                    lv = ln2p.tile([P, QB], F32, tag="l2", name=f"lv2_{qb}")
                    nc.scalar.activation(lv[:], var[:], AF.Ln, bias=eps_t[:])
                    iv = ln2sp.tile([P, QB], BF16, tag="iv2",
                                    name=f"iv2_{qb}")
                    ng = ln2sp.tile([P, QB], BF16, tag="ng2",
                                    name=f"ng2_{qb}")
                    with nc.allow_low_precision(reason="ln scale bf16"):
                        nc.scalar.activation(iv[:], lv[:], AF.Exp, scale=-0.5)
                        nc.vector.scalar_tensor_tensor(
                            ng[:], mu[:], -1.0, iv[:],
                            op0=OP.mult, op1=OP.mult)
                    ln2_sc[qb] = (iv, ng)
                    DBG[f"iv2_{qb}"] = iv
                    DBG[f"ng2_{qb}"] = ng
                    DBG[f"mu2_{qb}"] = mu
                    DBG[f"var2_{qb}"] = var
                th.append(ln2_fin)

                def x2n_w(half):
                    def f():
                        iv, ng = ln2_sc[qb]
                        for k in range(4 * half, 4 * half + 4):
                            t_ = x2tp.tile([P, QB], BF16, tag="x2t",
                                           name=f"xnt{qb}_{k}")
                            nc.gpsimd.tensor_mul(t_[:], x2[:, k, ts(qb, QB)],
                                                 iv[:])
                            with nc.allow_low_precision(reason="x2n bf16"):
                                nc.gpsimd.tensor_add(x2n[:, k, ts(qb, QB)],
                                                     t_[:], ng[:])
                    return f
                th.append(x2n_w(0))
                th.append(x2n_w(1))

                def fc1_grp(g):
                    def f():
                        ps = psF.tile([P, 4, QB], F32, tag="pf",
                                      name=f"p1_{qb}_{g}")
                        for fi in range(4):
                            fch = 4 * g + fi
                            for k in range(NCK):
                                nc.tensor.matmul(
                                    ps[:, fi, :], w1R[:, fch, k, :],
                                    x2n[:, k, ts(qb, QB)],
                                    start=(k == 0), stop=(k == NCK - 1),
                                    skip_group_check=True)
                        # drain psum -> h1T (gelu deferred to tail);
                        # GPSIMD can't read PSUM: alternate DVE/ACT, but
                        # keep qb3's drains off ACT so the chased gelus
                        # don't serialize the single psF slot
                        if qb == NQB - 1 or g % 2 == 0:
                            nc.vector.tensor_scalar_mul(
                                h1T[:, 4 * g:4 * g + 4, ts(qb, QB)],
                                ps[:], 1.0)
                        else:
                            nc.scalar.copy(
                                h1T[:, 4 * g:4 * g + 4, ts(qb, QB)], ps[:])
                        if qb == NQB - 1:
                            # last block: gelu chases fc1 so fc2 can stream
                            for fi in range(4):
                                fch = 4 * g + fi
                                nc.scalar.activation(
                                    h1T[:, fch, :], h1T[:, fch, :], AF.Gelu,
                                    bias=b1_t[:, fch:fch + 1])
                    return f
                f1 = [fc1_grp(g) for g in range(NFF // 4)]
                return th, f1

            pend = []
            fc1s = {}
            for qb in range(NQB):
                for h in range(H):
                    e = h % 2
                    if qb == 0 and h in pt_pre:
                        pt = pt_pre.pop(h)
                    else:
                        pt = sc_exp(qb, h)
                    ov = psO.tile([HD + 1, QB], F32, tag="ov",
                                  name=f"ov{qb}_{h}")
                    for a in range(4):
                        nc.tensor.matmul(ov[:], vE[:, a, :, h, :],
                                         pt[:, 2 * a:2 * a + 2, :],
                                         start=(a == 0), stop=(a == 3),
                                         perf_mode=DR, skip_group_check=True)
                    rc = rcp.tile([1, QB], BF16, tag="rc",
                                  name=f"rc{qb}_{h}")
                    with nc.allow_low_precision(reason="softmax denom bf16"):
                        nc.vector.reciprocal(rc[:], ov[64:65, :])
                    # stage ov into oE (SBUF) first: walrus allows only one
                    # PSUM input per DVE op, so the bc multiply is in-place
                    ch = h // 2
                    with nc.allow_low_precision(reason="fp8 oE"):
                        if e == 0:
                            nc.vector.tensor_scalar_mul(
                                oE[0:HD, ch, ts(qb, QB)], ov[0:HD, :],
                                1.0 / OESUB)
                        else:
                            nc.scalar.mul(oE[HD:P, ch, ts(qb, QB)],
                                          ov[0:HD, :], 1.0 / OESUB)
                    if e == 0:
                        ov_hold[0] = rc
                    else:
                        rc0 = ov_hold.pop(0)
                        bcp = psO.tile([P, QB], F32, tag="ov",
                                       name=f"bc{qb}_{ch}")
                        nc.tensor.matmul(bcp[0:HD, :], ones64[:], rc0[:],
                                         start=True, stop=True,
                                         skip_group_check=True)
                        nc.tensor.matmul(bcp[HD:P, :], ones64[:], rc[:],
                                         start=True, stop=True,
                                         skip_group_check=True)
                        with nc.allow_low_precision(reason="fp8 oE"):
                            nc.vector.tensor_mul(
                                oE[0:HD, ch, ts(qb, QB)],
                                oE[0:HD, ch, ts(qb, QB)], bcp[0:HD, :])
                            nc.vector.tensor_mul(
                                oE[HD:P, ch, ts(qb, QB)],
                                oE[HD:P, ch, ts(qb, QB)], bcp[HD:P, :])
                    # interleave one pending MLP unit per head slot
                    if pend:
                        pend.pop(0)()
                for t_ in pend:
                    t_()
                fr, f1 = mlpa_thunks(qb)
                # window qb+1 interleave: two ready fc1(qb-1) units first
                # (cover the proj->oE wait at the window boundary), then
                # alternate the serial front(qb) chain with fc1 so front
                # still finishes mid-window for the next window's fc1
                f1p = fc1s.get(qb - 1, [])
                mixed = list(f1p[:2])
                rest = list(f1p[2:])
                for i, t in enumerate(fr):
                    mixed.append(t)
                    if i < len(rest):
                        mixed.append(rest[i])
                mixed.extend(rest[len(fr):])
                pend = mixed
                fc1s[qb] = f1
            for t_ in pend:
                t_()
            for t_ in fc1s[NQB - 1]:
                t_()

            # =============== tail: fc2 (gelu already chased fc1) ===============
            for i in range(4):
                ps = psA.tile([P, 1024], F32, tag="ps", name=f"psf2_{i}")
                for half in range(2):
                    o = 2 * i + half
                    for fh in range(2):
                        w2t = w2s.tile([P, NFF // 2, P], BF16, tag="w2f",
                                       name=f"w2_{o}_{fh}")
                        nc.sync.dma_start(
                            w2t[:], w2_d[o][:, fh * 2048:(fh + 1) * 2048])
                        tpe(w2t[0:1, 0, 0:1])
                        for fi in range(NFF // 2):
                            f = fh * (NFF // 2) + fi
                            nc.tensor.matmul(
                                ps[:, ts(half, 512)], w2t[:, fi, :],
                                h1T[:, f, :],
                                start=(f == 0), stop=(f == NFF - 1),
                                skip_group_check=True)
                for half in range(2):
                    o = 2 * i + half
                    outt = outts.tile([P, MT], BF16, tag="outt",
                                      name=f"out{o}")
                    with nc.allow_low_precision(reason="bf16 output"):
                        nc.vector.scalar_tensor_tensor(
                            outt[:], ps[:, ts(half, 512)], b2_t[:, o:o + 1],
                            x2[:, o, :], op0=OP.add, op1=OP.add)
                    if o % 2 == 0:
                        nc.sync.dma_start(out_d[:, ts(o, 512)], outt[:])
                    else:
                        nc.gpsimd.dma_start(out_d[:, ts(o, 512)], outt[:])

    nc.compile()
    return nc


# ---------------- host side ----------------

def _bf16(a):
    return np.ascontiguousarray(a.astype(ml_dtypes.bfloat16))


def _f32(a):
    return np.ascontiguousarray(a.astype(np.float32))


def _fp8(a):
    return np.ascontiguousarray(
        np.clip(a, -240.0, 240.0).astype(ml_dtypes.float8_e4m3))


def _qk_perm():
    """out-channel permutation: chunk c = lohi*4+hg, partition p = b*32+r
    holds orig channel 64*(4*hg+b) + 32*lohi + r."""
    perm = np.empty(C, np.int64)
    for c in range(NCK):
        lohi, hg = c // 4, c % 4
        for p in range(P):
            b_, r = p // 32, p % 32
            perm[c * P + p] = 64 * (4 * hg + b_) + 32 * lohi + r
    return perm


def _dr_pack(W):
    """[out (nck*128), in C] -> [nck, P, (t, j, m)] DoubleRow layout:
    element [c][p][t, j, m] = W[c*128+m, (2t+j)*128+p]."""
    nck = W.shape[0] // P
    Wr = W.reshape(nck, P, NCK, P)          # [c, m, kin, p]
    out = np.empty((nck, P, 4, 2, P), W.dtype)
    for t in range(4):
        for j in range(2):
            out[:, :, t, j, :] = Wr[:, :, 2 * t + j, :].transpose(0, 2, 1)
    return out.reshape(nck, P, C)


def prepare_inputs(x, qkv_w, qkv_b, attn_proj_w, attn_proj_b, blk_proj_w,
                   blk_proj_b, ln1_g, ln1_b, ln2_g, ln2_b, fc1_w, fc1_b,
                   fc2_w, fc2_b, mask):
    x = np.asarray(x, np.float32)
    qkv_w = np.asarray(qkv_w, np.float64)
    qkv_b = np.asarray(qkv_b, np.float64)

    g1 = np.asarray(ln1_g, np.float64)
    bl1 = np.asarray(ln1_b, np.float64)
    Wq = qkv_w[0:C] * g1[None, :]
    bq = qkv_w[0:C] @ bl1 + qkv_b[0:C]
    Wk = qkv_w[C:2 * C] * g1[None, :]
    bk = qkv_w[C:2 * C] @ bl1 + qkv_b[C:2 * C]
    Wv = qkv_w[2 * C:] * g1[None, :]
    bv = qkv_w[2 * C:] @ bl1 + qkv_b[2 * C:]

    A = np.asarray(attn_proj_w, np.float64)
    Bw = np.asarray(blk_proj_w, np.float64)
    Wm = Bw @ A
    bm = Wm @ bv + Bw @ np.asarray(attn_proj_b, np.float64) \
        + np.asarray(blk_proj_b, np.float64)

    g2 = np.asarray(ln2_g, np.float64)
    bl2 = np.asarray(ln2_b, np.float64)
    W1 = np.asarray(fc1_w, np.float64) * g2[None, :]
    b1 = np.asarray(fc1_w, np.float64) @ bl2 + np.asarray(fc1_b, np.float64)
    W2 = np.asarray(fc2_w, np.float64)
    b2 = np.asarray(fc2_b, np.float64)

    perm = _qk_perm()
    wq_l = _fp8(_dr_pack((SW * Wq)[perm]))
    wk_l = _fp8(_dr_pack((SW * Wk)[perm]))
    bqP = (SW * bq)[perm]
    bkP = (SW * bk)[perm]
    # V: [vb][t][p][(j, n)]: SW * Wv[vb*512+n, (2t+j)*128+p]
    WvS = (SW * Wv).reshape(2, 512, NCK, P)     # [vb, n, kin, p]
    wv_l = np.empty((2, 4, P, 2, 512), np.float64)
    for t in range(4):
        for j in range(2):
            wv_l[:, t, :, j, :] = WvS[:, :, 2 * t + j, :].transpose(0, 2, 1)
    wv_l = _fp8(wv_l.reshape(2, 4, P, 1024))
    wm_l = _fp8(_dr_pack(SW * Wm))
    w1_l = _bf16(W1.reshape(NFF, P, NCK, P).transpose(0, 3, 2, 1)
                 .reshape(NFF, P, C))
    w2_l = _bf16(W2.reshape(NCK, P, NFF, P).transpose(0, 3, 2, 1)
                 .reshape(NCK, P, FF))
    bqk_l = _f32(np.concatenate([bqP.reshape(NCK, P).T,
                                 bkP.reshape(NCK, P).T], axis=1))
    bm_l = _f32(bm.reshape(NCK, P).T)
    b1_l = _f32(b1.reshape(NFF, P).T)
    b2_l = _f32(b2.reshape(NCK, P).T)

    shared = dict(wq=wq_l, wk=wk_l, wv=wv_l, wm=wm_l, w1=w1_l, w2=w2_l,
                  bqk=bqk_l, bm=bm_l, b1=b1_l, b2=b2_l)

    in_maps = []
    for core in range(8):
        b_, m = divmod(core, 2)
        xb = x[b_]
        xp = np.concatenate([xb[m * MT:(m + 1) * MT],
                             xb[(1 - m) * MT:(2 - m) * MT]], axis=0)
        xt_l = _bf16(xp.reshape(NT, NCK, P).transpose(2, 1, 0)
                     .reshape(P, NCK * NT))
        in_maps.append(dict(shared, xt=xt_l))
    return in_maps


def gather_output(results):
    out = np.empty((B, N, C), np.float32)
    for core in range(8):
        b_, m = divmod(core, 2)
        O = np.asarray(results[core]["outT"]).astype(np.float32)
        O = O.reshape(P, NCK, MT)
        out[b_, m * MT:(m + 1) * MT, :] = O.transpose(2, 1, 0).reshape(MT, C)
    return out


_CACHE = {}


def kernel(**inputs):
    if "nc" not in _CACHE:
        _CACHE["nc"] = build_module()
    nc = _CACHE["nc"]
    in_maps = prepare_inputs(**inputs)
    res = run_bass_kernel_spmd(nc, in_maps, core_ids=list(range(8)))
    return gather_output(res.results)


# revision 15
# speedup vs baseline: 1.2851x; 1.0028x over previous
"""Trainium2 Bass kernel for nn_Block_44358422233377 (dense transformer block).

v2: fp8e4m3+DoubleRow attention side (4x cheaper per MAC in the cost model),
bf16 MLP, 4-deep query-block software pipeline overlapping the ACT-bound
softmax-exp with PE-bound MLP-front work, single-pass x streaming, DMA issue
split across SP (x, attn weights, w2, out) and Pool (w1, fc1-psum drains).

Sharding: core c = (batch b = c//2, query-half m = c%2); K/V recomputed per
sibling (no collectives). Activations live transposed [channels(part), tok].

Numerics: attn weights *32 -> e4m3; scores psum = 1024*s_true; softmax via
exp(s_raw/8192 - 2) in e4m3 (denominator via the ones-column of V; the
common shift cancels in the normalize). oE staged as ov/512 in fp8, then
scaled in place by the PE-broadcast 2048*RECS'/denom; merged-proj descale
1/(SW*SW*RECS) folded into the x2 write. MLP stays bf16 (fp8 fails the
2e-2 gate). Output DMA'd bf16, upcast on host. End-to-end rel err 7.4e-3.

Walrus BIR rules honored (the CoreSim-only version broke all three):
GPSIMD never touches PSUM; DVE/ACT ops read at most one PSUM operand;
two SBUF inputs of a DVE op share a partition base. Interleaved matmul
accumulation chains never share a 2KB PSUM zero region (pending-zero
re-marking silently zeroes the neighbor chain's partials).
"""
import sys

sys.path.insert(0, "/opt/trn_rl_repo")

import numpy as np
import ml_dtypes

import concourse.bass as bass
import concourse.bacc as bacc
import concourse.mybir as mybir
import concourse.tile as tile
from concourse.bass import ts
from concourse.bass_utils import run_bass_kernel_spmd

F32 = mybir.dt.float32
BF16 = mybir.dt.bfloat16
FP8 = mybir.dt.float8e4
AF = mybir.ActivationFunctionType
OP = mybir.AluOpType
DR = mybir.MatmulPerfMode.DoubleRow

P = 128
B, N, C, H = 4, 1024, 1024, 16
HD = C // H          # 64
FF = 4 * C           # 4096
NT = N               # context tokens per core
MT = N // 2          # own (query) tokens per core
QB = 128             # query sub-block (pipeline granularity)
NQB = MT // QB       # 4
EPS = 1e-6
NCK = C // P         # 8 channel chunks
NFF = FF // P        # 32 ff chunks
SW = 32.0            # fp8 weight scale
EXPS = 1.0 / (8.0 * SW * SW)       # exp scale  (= 1/8192)
RECS = 4.0                         # oE scale (vs o_true: SW*RECS)
OESUB = 512.0                      # staging scale: oE_pre = ov/OESUB
PROJS = 1.0 / (SW * SW * RECS)     # proj psum descale (= 1/16384)


DBG = {}


def build_module():
    nc = bacc.Bacc("TRN2", target_bir_lowering=False, debug=False)

    xt_d = nc.dram_tensor("xt", [P, NCK * NT], BF16, kind="ExternalInput")
    wq_d = nc.dram_tensor("wq", [NCK, P, C], FP8, kind="ExternalInput")
    wk_d = nc.dram_tensor("wk", [NCK, P, C], FP8, kind="ExternalInput")
    wv_d = nc.dram_tensor("wv", [2, 4, P, 1024], FP8, kind="ExternalInput")
    wm_d = nc.dram_tensor("wm", [NCK, P, C], FP8, kind="ExternalInput")
    w1_d = nc.dram_tensor("w1", [NFF, P, C], BF16, kind="ExternalInput")
    w2_d = nc.dram_tensor("w2", [NCK, P, FF], BF16, kind="ExternalInput")
    bqk_d = nc.dram_tensor("bqk", [P, 16], F32, kind="ExternalInput")
    bm_d = nc.dram_tensor("bm", [P, NCK], F32, kind="ExternalInput")
    b1_d = nc.dram_tensor("b1", [P, NFF], F32, kind="ExternalInput")
    b2_d = nc.dram_tensor("b2", [P, NCK], F32, kind="ExternalInput")
    out_d = nc.dram_tensor("outT", [P, NCK * MT], BF16, kind="ExternalOutput")

    wv_tiles = {}
    wm_tiles = {}
    ln2_ps = {}
    ln2_sc = {}
    ov_hold = {}

    with tile.TileContext(nc) as tc:
        with (
            tc.tile_pool(name="const", bufs=1) as cpool,
            tc.tile_pool(name="persist", bufs=1) as big,
            tc.tile_pool(name="sc", bufs=4) as sc,
            tc.tile_pool(name="sq", bufs=2) as sqp,
            tc.tile_pool(name="tmpb", bufs=2) as tmpp,
            tc.tile_pool(name="x2t", bufs=2) as x2tp,
            tc.tile_pool(name="ln2", bufs=4) as ln2p,
            tc.tile_pool(name="ln2s", bufs=2) as ln2sp,
            tc.tile_pool(name="wblk", bufs=16) as wblk,
            tc.tile_pool(name="w2s", bufs=2) as w2s,
            tc.tile_pool(name="pt", bufs=3) as ptp,
            tc.tile_pool(name="rc", bufs=2) as rcp,
            tc.tile_pool(name="outts", bufs=2) as outts,
            tc.tile_pool(name="psA", bufs=2, space="PSUM") as psA,
            tc.tile_pool(name="psF", bufs=1, space="PSUM") as psF,
            tc.tile_pool(name="psO", bufs=3, space="PSUM") as psO,
        ):
            # ---- constants / biases ----
            ones128 = cpool.tile([P, P], BF16, tag="ones128")
            nc.vector.memset(ones128[:], 1.0)
            ones64 = cpool.tile([1, HD], BF16, tag="ones64")
            nc.vector.memset(ones64[:], RECS * OESUB)
            eps_t = cpool.tile([P, 1], F32, tag="eps")
            nc.vector.memset(eps_t[:], EPS)
            nm2_t = cpool.tile([P, 1], F32, tag="nm2")
            nc.vector.memset(nm2_t[:], -2.0)
            dumv = cpool.tile([1, 8], F32, tag="dumv")
            bqk_t = cpool.tile([P, 16], F32, tag="bqk")
            bm_t = cpool.tile([P, NCK], F32, tag="bm")
            b1_t = cpool.tile([P, NFF], F32, tag="b1")
            b2_t = cpool.tile([P, NCK], F32, tag="b2")

            def tdve(ap):
                """Absorb a DMA's semaphore onto the DVE clock."""
                nc.vector.tensor_max(dumv[0:1, 0:1], ap, ap)

            def tpe(ap):
                """Absorb a weight-DMA's semaphore onto the PE clock."""
                nc.tensor.ldweights(ap)

            # ---- persistent activations ----
            xt = big.tile([P, NCK, NT], BF16, tag="xt")
            xnT = big.tile([P, NCK, NT], FP8, tag="xnT")
            kE = big.tile([P, 2, 4, NT], FP8, tag="kE")
            qE = big.tile([P, 2, 4, MT], FP8, tag="qE")
            vE = big.tile([P, 4, 2, H, HD + 1], FP8, tag="vE")
            oE = big.tile([P, NCK, MT], FP8, tag="oE")
            x2 = big.tile([P, NCK, MT], BF16, tag="x2")
            x2n = big.tile([P, NCK, MT], BF16, tag="x2n")
            h1T = big.tile([P, NFF, MT], BF16, tag="h1T")
            w1R = big.tile([P, NFF, NCK, P], BF16, tag="w1R")

            inv1 = big.tile([P, 2, 512], BF16, tag="inv1")
            ngm1 = big.tile([P, 2, 512], BF16, tag="ngm1")
            DBG.update(xnT=xnT, kE=kE, qE=qE, vE=vE, oE=oE, x2=x2,
                       x2n=x2n, h1T=h1T, inv1=inv1, ngm1=ngm1)

            nc.vector.memset(vE[:, :, :, :, HD:HD + 1], 1.0)

            # wblk ring slot plan (16 bufs): wk 0-7, wq 8-15, wv 0-7 (after
            # K chains), wm 8-15 (after Q chains) — no cross-stream cycles.
            # wk tiles allocated first (ring order); DMAs issued on Pool
            # after the xt stream so LN1 stats aren't delayed.
            wk_tiles = {}
            for c in range(NCK):
                wk_tiles[c] = wblk.tile([P, 4, 2, P], FP8, tag="wblk",
                                        name=f"wk{c}")

            # PE p-state warm-up: ~3.5us of throwaway matmuls so the LN1
            # stats chains (and everything after) run at the 2.4GHz rate
            # instead of paying the mid/cold ramp on the critical path
            wup = psA.tile([P, 64], F32, tag="ps", name="warmup")
            for i in range(70):
                nc.tensor.matmul(wup[:, 0:64], ones128[:, 0:P],
                                 ones128[:, 0:64], start=True, stop=True,
                                 skip_group_check=True)

            # =============== LN1 stats (single x pass) ===============
            pssq = [psA.tile([P, 1024], F32, tag="ps", name=f"pssq{tb}")
                    for tb in range(2)]
            for k in range(NCK):
                if k % 2 == 0:
                    nc.sync.dma_start(xt[:, k, :], xt_d[:, ts(k, NT)])
                else:
                    nc.gpsimd.dma_start(xt[:, k, :], xt_d[:, ts(k, NT)])
                tdve(xt[0:1, k, 0:1])
                for tb in range(2):
                    sq = sqp.tile([P, 512], BF16, tag="sq",
                                  name=f"sqB{k}_{tb}")
                    with nc.allow_low_precision(reason="ln stats bf16"):
                        nc.scalar.square(sq[:], xt[:, k, ts(tb, 512)])
                    nc.tensor.matmul(pssq[tb][:, 0:512], ones128[:],
                                     xt[:, k, ts(tb, 512)],
                                     start=(k == 0), stop=(k == NCK - 1),
                                     skip_group_check=True)
                    nc.tensor.matmul(pssq[tb][:, 512:1024], ones128[:],
                                     sq[:],
                                     start=(k == 0), stop=(k == NCK - 1),
                                     skip_group_check=True)

            # weight/bias DMA issue, after xt so stats aren't stalled
            for c in range(NCK):
                nc.gpsimd.dma_start(wk_tiles[c][:], wk_d[c])
                tpe(wk_tiles[c][0:1, 0, 0, 0:1])
            nc.sync.dma_start(bqk_t[:], bqk_d[:])
            nc.sync.dma_start(bm_t[:], bm_d[:])
            nc.sync.dma_start(b1_t[:], b1_d[:])
            nc.sync.dma_start(b2_t[:], b2_d[:])

            for tb in range(2):
                mu = sc.tile([P, 512], BF16, tag="scb", name=f"mu1_{tb}")
                with nc.allow_low_precision(reason="ln stats bf16"):
                    nc.scalar.activation(mu[:], pssq[tb][:, 0:512], AF.Copy,
                                         scale=1.0 / C)
                musq = sc.tile([P, 512], BF16, tag="scb", name=f"musq1_{tb}")
                nc.vector.tensor_mul(musq[:], mu[:], mu[:])
                var = sc.tile([P, 512], BF16, tag="scb", name=f"var1_{tb}")
                with nc.allow_low_precision(reason="ln stats bf16"):
                    nc.vector.scalar_tensor_tensor(
                        var[:], pssq[tb][:, 512:1024], 1.0 / C, musq[:],
                        op0=OP.mult, op1=OP.subtract)
                std = sc.tile([P, 512], BF16, tag="scb", name=f"std1_{tb}")
                nc.scalar.activation(std[:], var[:], AF.Sqrt, bias=eps_t[:])
                with nc.allow_low_precision(reason="ln scale bf16"):
                    nc.vector.reciprocal(inv1[:, tb, :], std[:])
                    nc.vector.scalar_tensor_tensor(
                        ngm1[:, tb, :], mu[:], -1.0, inv1[:, tb, :],
                        op0=OP.mult, op1=OP.mult)

            # =============== LN1 apply (from SBUF) -> xnT fp8 ===============
            for k in range(NCK):
                for tb in range(2):
                    tmp = tmpp.tile([P, 512], BF16, tag="tmpb",
                                    name=f"lt{k}_{tb}")
                    nc.vector.tensor_mul(tmp[:], xt[:, k, ts(tb, 512)],
                                         inv1[:, tb, :])
                    with nc.allow_low_precision(reason="fp8 activations"):
                        nc.vector.tensor_add(xnT[:, k, ts(tb, 512)], tmp[:],
                                             ngm1[:, tb, :])

            # =============== Q / K projections (DoubleRow fp8) ===============
            # chunk c = lohi*4 + hg holds perm'd out-channels (see host prep)
            def qk_chain(ps_slice, w, qsl):
                for t in range(4):
                    nc.tensor.matmul(ps_slice, w[:, t, :, :],
                                     xnT[:, 2 * t:2 * t + 2, qsl],
                                     start=(t == 0), stop=(t == 3),
                                     perf_mode=DR, skip_group_check=True)

            for i in range(4):
                ps = psA.tile([P, 1024], F32, tag="ps", name=f"psq{i}")
                for half in range(2):
                    c = 2 * i + half
                    w = wblk.tile([P, 4, 2, P], FP8, tag="wblk",
                                  name=f"wq{c}")
                    nc.sync.dma_start(w[:], wq_d[c])
                    tpe(w[0:1, 0, 0, 0:1])
                    qk_chain(ps[:, ts(half, 512)], w, slice(0, MT))
                for half in range(2):
                    c = 2 * i + half
                    lohi, hg = c // 4, c % 4
                    with nc.allow_low_precision(reason="fp8 activations"):
                        nc.scalar.activation(
                            qE[:, lohi, hg, :], ps[:, ts(half, 512)],
                            AF.Identity, bias=bqk_t[:, c:c + 1])
            # wv on Pool (ring slots 0-7, reusing wk slots after K chains)
            for vb in range(2):
                for t in range(4):
                    w = wblk.tile([P, 2, 512], FP8, tag="wblk",
                                  name=f"wv{vb}_{t}")
                    nc.gpsimd.dma_start(w[:], wv_d[vb, t])
                    tpe(w[0:1, 0, 0:1])
                    wv_tiles[(vb, t)] = w
            # wm upfront on SP (slots 8-15 after wq), then w1 resident on SP
            for o in range(NCK):
                w = wblk.tile([P, 4, 2, P], FP8, tag="wblk", name=f"wm{o}")
                nc.sync.dma_start(w[:], wm_d[o])
                tpe(w[0:1, 0, 0, 0:1])
                wm_tiles[o] = w
            for f in range(NFF):
                nc.sync.dma_start(w1R[:, f, :, :], w1_d[f])
            tdve(w1R[0:1, 0, 0, 0:1])
            # K chains ordered so head-group hg's chunks (hg, hg+4) finish
            # first, letting window-0 scores start while K still runs
            for c in [0, 4, 1, 5, 2, 6, 3, 7]:
                ps = psA.tile([P, NT], F32, tag="ps", name=f"psk{c}")
                for tb in range(2):
                    qk_chain(ps[:, ts(tb, 512)], wk_tiles[c],
                             slice(tb * 512, tb * 512 + 512))
                lohi, hg = c // 4, c % 4
                with nc.allow_low_precision(reason="fp8 activations"):
                    nc.scalar.activation(kE[:, lohi, hg, :], ps[:],
                                         AF.Identity,
                                         bias=bqk_t[:, NCK + c:NCK + c + 1])

            # =============== V projection (DoubleRow fp8) ===============
            # out [128 tok, 512 vd] per (tok-chunk t8, vb); vE gets v_hat=32v
            pt_pre = {}

            def sc_exp(qb, h):
                b_, hg = h % 4, h // 4
                sp = psA.tile([P, NCK, QB], F32, tag="ps",
                              name=f"sp{qb}_{h}")
                for kc in range(NCK):
                    nc.tensor.matmul(
                        sp[:, kc, :],
                        kE[32 * b_:32 * b_ + 32, :, hg, ts(kc, P)],
                        qE[32 * b_:32 * b_ + 32, :, hg, ts(qb, QB)],
                        start=True, stop=True, perf_mode=DR,
                        skip_group_check=True,
                        tile_position=(32 * b_, 0))
                pt = ptp.tile([P, NCK, QB], FP8, tag="pt",
                              name=f"pt{qb}_{h}")
                with nc.allow_low_precision(reason="fp8 exp scores"):
                    nc.scalar.activation(pt[:], sp[:], AF.Exp,
                                         bias=nm2_t[:], scale=EXPS)
                return pt

            for t8 in range(NCK):
                ps = psA.tile([P, 1024], F32, tag="ps", name=f"psv{t8}")
                for vb in range(2):
                    for t in range(4):
                        nc.tensor.matmul(
                            ps[:, ts(vb, 512)],
                            xnT[:, 2 * t:2 * t + 2, ts(t8, P)],
                            wv_tiles[(vb, t)][:],
                            start=(t == 0), stop=(t == 3),
                            perf_mode=DR, skip_group_check=True)
                jg, pr = t8 // 2, t8 % 2
                for vb in range(2):
                    # spread the drain ops over DVE and ACT so neither
                    # serial queue gates the first PV (GPSIMD can't read
                    # PSUM per the BIR verifier)
                    with nc.allow_low_precision(reason="fp8 acts"):
                        if (2 * t8 + vb) % 2 == 0:
                            nc.vector.tensor_scalar_mul(
                                vE[:, jg, pr, ts(vb, 8), 0:HD],
                                ps[:, ts(vb, 512)].rearrange(
                                    "p (h d) -> p h d", d=HD), 1.0)
                        else:
                            nc.scalar.copy(
                                vE[:, jg, pr, ts(vb, 8), 0:HD],
                                ps[:, ts(vb, 512)].rearrange(
                                    "p (h d) -> p h d", d=HD))
                if t8 >= 6:
                    # scores/exp for window-0's first heads overlap the V
                    # tail (scores never read vE, so no PV deadlock)
                    pt_pre[t8 - 6] = sc_exp(0, t8 - 6)

            # =============== pipelined attention + MLP-front ===============
            def mlpa_thunks(qb):
                """proj+LN2+fc1 work units for query block qb (deps in
                order); emitted interleaved with attention of block qb+1."""
                th = []

                def proj_half(hf):
                    def f():
                        ps = psF.tile([P, 4, QB], F32, tag="pf",
                                      name=f"pm{qb}_{hf}")
                        for o in range(4 * hf, 4 * hf + 4):
                            wt = wm_tiles[o]
                            for t in range(4):
                                nc.tensor.matmul(
                                    ps[:, o - 4 * hf, :], wt[:, t, :, :],
                                    oE[:, 2 * t:2 * t + 2, ts(qb, QB)],
                                    start=(t == 0), stop=(t == 3),
                                    perf_mode=DR, skip_group_check=True)
                        for o in range(4 * hf, 4 * hf + 4):
                            t_ = x2tp.tile([P, QB], BF16, tag="x2t",
                                           name=f"x2t{qb}_{o}")
                            nc.vector.tensor_scalar(
                                t_[:], ps[:, o - 4 * hf, :], PROJS,
                                bm_t[:, o:o + 1], op0=OP.mult, op1=OP.add)
                            with nc.allow_low_precision(reason="x2 bf16"):
                                nc.vector.tensor_add(
                                    x2[:, o, ts(qb, QB)], t_[:],
                                    xt[:, o, qb * QB:qb * QB + QB])
                    return f
                th.append(proj_half(0))
                th.append(proj_half(1))

                def ln2_stats():
                    # sequential chains (sq first, then x): interleaved
                    # chains in one 2KB zero region corrupt each other via
                    # pending-zero re-marking; sequential chains are safe
                    psa = psO.tile([P, 4, QB], F32, tag="ov",
                                   name=f"pl2_{qb}")
                    ln2_ps[qb] = psa
                    for k in range(NCK):
                        sq2 = sqp.tile([P, QB], BF16, tag="sq",
                                       name=f"sq2_{qb}_{k}")
                        nc.gpsimd.tensor_mul(sq2[:], x2[:, k, ts(qb, QB)],
                                             x2[:, k, ts(qb, QB)])
                        nc.tensor.matmul(psa[:, 1, :], ones128[:], sq2[:],
                                         start=(k == 0), stop=(k == NCK - 1),
                                         skip_group_check=True)
                    for k in range(NCK):
                        nc.tensor.matmul(psa[:, 0, :], ones128[:],
                                         x2[:, k, ts(qb, QB)],
                                         start=(k == 0), stop=(k == NCK - 1),
                                         skip_group_check=True)
                th.append(ln2_stats)

                def ln2_fin():
                    psa = ln2_ps.pop(qb)
                    psb = psa[:, 1:2, :]
                    mu = ln2p.tile([P, QB], F32, tag="l2", name=f"mu2_{qb}")
                    nc.scalar.activation(mu[:], psa[:, 0, :], AF.Copy,
                                         scale=1.0 / C)
                    musq = ln2p.tile([P, QB], F32, tag="l2",
                                     name=f"msq2_{qb}")
                    nc.vector.tensor_mul(musq[:], mu[:], mu[:])
                    var = ln2p.tile([P, QB], F32, tag="l2", name=f"var2_{qb}")
                    nc.vector.scalar_tensor_tensor(
                        var[:], psb[:, 0, :], 1.0 / C, musq[:],
                        op0=OP.mult, op1=OP.subtract)
                    # inv-std = exp(-0.5*ln(var+eps)): Ln and Exp share an
                    # ACT table, so no table switch amid the exp stream
                    lv = ln2p.tile([P, QB], F32, tag="l2", name=f"lv2_{qb}")
                    nc.scalar.activation(lv[:], var[:], AF.Ln, bias=eps_t[:])
                    iv = ln2sp.tile([P, QB], BF16, tag="iv2",
                                    name=f"iv2_{qb}")
                    ng = ln2sp.tile([P, QB], BF16, tag="ng2",
                                    name=f"ng2_{qb}")
                    with nc.allow_low_precision(reason="ln scale bf16"):
                        nc.scalar.activation(iv[:], lv[:], AF.Exp, scale=-0.5)
                        nc.vector.scalar_tensor_tensor(
                            ng[:], mu[:], -1.0, iv[:],
                            op0=OP.mult, op1=OP.mult)
                    ln2_sc[qb] = (iv, ng)
                    DBG[f"iv2_{qb}"] = iv
                    DBG[f"ng2_{qb}"] = ng
                    DBG[f"mu2_{qb}"] = mu
                    DBG[f"var2_{qb}"] = var
                th.append(ln2_fin)

                def x2n_w(half):
                    def f():
                        iv, ng = ln2_sc[qb]
                        for k in range(4 * half, 4 * half + 4):
                            t_ = x2tp.tile([P, QB], BF16, tag="x2t",
                                           name=f"xnt{qb}_{k}")
                            nc.gpsimd.tensor_mul(t_[:], x2[:, k, ts(qb, QB)],
                                                 iv[:])
                            with nc.allow_low_precision(reason="x2n bf16"):
                                nc.gpsimd.tensor_add(x2n[:, k, ts(qb, QB)],
                                                     t_[:], ng[:])
                    return f
                th.append(x2n_w(0))
                th.append(x2n_w(1))

                def fc1_grp(g):
                    def f():
                        ps = psF.tile([P, 4, QB], F32, tag="pf",
                                      name=f"p1_{qb}_{g}")
                        for fi in range(4):
                            fch = 4 * g + fi
                            for k in range(NCK):
                                nc.tensor.matmul(
                                    ps[:, fi, :], w1R[:, fch, k, :],
                                    x2n[:, k, ts(qb, QB)],
                                    start=(k == 0), stop=(k == NCK - 1),
                                    skip_group_check=True)
                        # drain psum -> h1T (gelu deferred to tail);
                        # GPSIMD can't read PSUM: alternate DVE/ACT, but
                        # keep qb3's drains off ACT so the chased gelus
                        # don't serialize the single psF slot
                        if qb == NQB - 1 or g % 2 == 0:
                            nc.vector.tensor_scalar_mul(
                                h1T[:, 4 * g:4 * g + 4, ts(qb, QB)],
                                ps[:], 1.0)
                        else:
                            nc.scalar.copy(
                                h1T[:, 4 * g:4 * g + 4, ts(qb, QB)], ps[:])
                        if qb == NQB - 1:
                            # last block: gelu chases fc1 so fc2 can stream
                            for fi in range(4):
                                fch = 4 * g + fi
                                nc.scalar.activation(
                                    h1T[:, fch, :], h1T[:, fch, :], AF.Gelu,
                                    bias=b1_t[:, fch:fch + 1])
                    return f
                f1 = [fc1_grp(g) for g in range(NFF // 4)]
                return th, f1

            pend = []
            fc1s = {}
            for qb in range(NQB):
                for h in range(H):
                    e = h % 2
                    if qb == 0 and h in pt_pre:
                        pt = pt_pre.pop(h)
                    else:
                        pt = sc_exp(qb, h)
                    ov = psO.tile([HD + 1, QB], F32, tag="ov",
                                  name=f"ov{qb}_{h}")
                    for a in range(4):
                        nc.tensor.matmul(ov[:], vE[:, a, :, h, :],
                                         pt[:, 2 * a:2 * a + 2, :],
                                         start=(a == 0), stop=(a == 3),
                                         perf_mode=DR, skip_group_check=True)
                    rc = rcp.tile([1, QB], BF16, tag="rc",
                                  name=f"rc{qb}_{h}")
                    with nc.allow_low_precision(reason="softmax denom bf16"):
                        nc.vector.reciprocal(rc[:], ov[64:65, :])
                    # stage ov into oE (SBUF) first: walrus allows only one
                    # PSUM input per DVE op, so the bc multiply is in-place
                    ch = h // 2
                    with nc.allow_low_precision(reason="fp8 oE"):
                        if e == 0:
                            nc.vector.tensor_scalar_mul(
                                oE[0:HD, ch, ts(qb, QB)], ov[0:HD, :],
                                1.0 / OESUB)
                        else:
                            nc.scalar.mul(oE[HD:P, ch, ts(qb, QB)],
                                          ov[0:HD, :], 1.0 / OESUB)
                    if e == 0:
                        ov_hold[0] = rc
                    else:
                        rc0 = ov_hold.pop(0)
                        bcp = psO.tile([P, QB], F32, tag="ov",
                                       name=f"bc{qb}_{ch}")
                        nc.tensor.matmul(bcp[0:HD, :], ones64[:], rc0[:],
                                         start=True, stop=True,
                                         skip_group_check=True)
                        nc.tensor.matmul(bcp[HD:P, :], ones64[:], rc[:],
                                         start=True, stop=True,
                                         skip_group_check=True)
                        with nc.allow_low_precision(reason="fp8 oE"):
                            nc.vector.tensor_mul(
                                oE[0:HD, ch, ts(qb, QB)],
                                oE[0:HD, ch, ts(qb, QB)], bcp[0:HD, :])
                            nc.vector.tensor_mul(
                                oE[HD:P, ch, ts(qb, QB)],
                                oE[HD:P, ch, ts(qb, QB)], bcp[HD:P, :])
                    # interleave one pending MLP unit per head slot
                    if pend:
                        pend.pop(0)()
                for t_ in pend:
                    t_()
                fr, f1 = mlpa_thunks(qb)
                # window qb+1 interleave: two ready fc1(qb-1) units first
                # (cover the proj->oE wait at the window boundary), then
                # alternate the serial front(qb) chain with fc1 so front
                # still finishes mid-window for the next window's fc1
                f1p = fc1s.get(qb - 1, [])
                mixed = list(f1p[:2])
                rest = list(f1p[2:])
                for i, t in enumerate(fr):
                    mixed.append(t)
                    if i < len(rest):
                        mixed.append(rest[i])
                mixed.extend(rest[len(fr):])
                pend = mixed
                fc1s[qb] = f1
            for t_ in pend:
                t_()
            for t_ in fc1s[NQB - 1]:
                t_()

            # =============== tail: fc2 (gelu already chased fc1) ===============
            for i in range(4):
                ps = psA.tile([P, 1024], F32, tag="ps", name=f"psf2_{i}")
                for half in range(2):
                    o = 2 * i + half
                    for fh in range(2):
                        w2t = w2s.tile([P, NFF // 2, P], BF16, tag="w2f",
                                       name=f"w2_{o}_{fh}")
                        nc.sync.dma_start(
                            w2t[:], w2_d[o][:, fh * 2048:(fh + 1) * 2048])
                        tpe(w2t[0:1, 0, 0:1])
                        for fi in range(NFF // 2):
                            f = fh * (NFF // 2) + fi
                            nc.tensor.matmul(
                                ps[:, ts(half, 512)], w2t[:, fi, :],
                                h1T[:, f, :],
                                start=(f == 0), stop=(f == NFF - 1),
                                skip_group_check=True)
                for half in range(2):
                    o = 2 * i + half
                    outt = outts.tile([P, MT], BF16, tag="outt",
                                      name=f"out{o}")
                    with nc.allow_low_precision(reason="bf16 output"):
                        nc.vector.scalar_tensor_tensor(
                            outt[:], ps[:, ts(half, 512)], b2_t[:, o:o + 1],
                            x2[:, o, :], op0=OP.add, op1=OP.add)
                    if o % 2 == 0:
                        nc.sync.dma_start(out_d[:, ts(o, 512)], outt[:])
                    else:
                        nc.gpsimd.dma_start(out_d[:, ts(o, 512)], outt[:])

    nc.compile()
    return nc


# ---------------- host side ----------------

def _bf16(a):
    return np.ascontiguousarray(a.astype(ml_dtypes.bfloat16))


def _f32(a):
    return np.ascontiguousarray(a.astype(np.float32))


def _fp8(a):
    return np.ascontiguousarray(
        np.clip(a, -240.0, 240.0).astype(ml_dtypes.float8_e4m3))


def _qk_perm():
    """out-channel permutation: chunk c = lohi*4+hg, partition p = b*32+r
    holds orig channel 64*(4*hg+b) + 32*lohi + r."""
    perm = np.empty(C, np.int64)
    for c in range(NCK):
        lohi, hg = c // 4, c % 4
        for p in range(P):
            b_, r = p // 32, p % 32
            perm[c * P + p] = 64 * (4 * hg + b_) + 32 * lohi + r
    return perm


def _dr_pack(W):
    """[out (nck*128), in C] -> [nck, P, (t, j, m)] DoubleRow layout:
    element [c][p][t, j, m] = W[c*128+m, (2t+j)*128+p]."""
    nck = W.shape[0] // P
    Wr = W.reshape(nck, P, NCK, P)          # [c, m, kin, p]
    out = np.empty((nck, P, 4, 2, P), W.dtype)
    for t in range(4):
        for j in range(2):
            out[:, :, t, j, :] = Wr[:, :, 2 * t + j, :].transpose(0, 2, 1)
    return out.reshape(nck, P, C)


def prepare_inputs(x, qkv_w, qkv_b, attn_proj_w, attn_proj_b, blk_proj_w,
                   blk_proj_b, ln1_g, ln1_b, ln2_g, ln2_b, fc1_w, fc1_b,
                   fc2_w, fc2_b, mask):
    x = np.asarray(x, np.float32)
    qkv_w = np.asarray(qkv_w, np.float64)
    qkv_b = np.asarray(qkv_b, np.float64)

    g1 = np.asarray(ln1_g, np.float64)
    bl1 = np.asarray(ln1_b, np.float64)
    Wq = qkv_w[0:C] * g1[None, :]
    bq = qkv_w[0:C] @ bl1 + qkv_b[0:C]
    Wk = qkv_w[C:2 * C] * g1[None, :]
    bk = qkv_w[C:2 * C] @ bl1 + qkv_b[C:2 * C]
    Wv = qkv_w[2 * C:] * g1[None, :]
    bv = qkv_w[2 * C:] @ bl1 + qkv_b[2 * C:]

    A = np.asarray(attn_proj_w, np.float64)
    Bw = np.asarray(blk_proj_w, np.float64)
    Wm = Bw @ A
    bm = Wm @ bv + Bw @ np.asarray(attn_proj_b, np.float64) \
        + np.asarray(blk_proj_b, np.float64)

    g2 = np.asarray(ln2_g, np.float64)
    bl2 = np.asarray(ln2_b, np.float64)
    W1 = np.asarray(fc1_w, np.float64) * g2[None, :]
    b1 = np.asarray(fc1_w, np.float64) @ bl2 + np.asarray(fc1_b, np.float64)
    W2 = np.asarray(fc2_w, np.float64)
    b2 = np.asarray(fc2_b, np.float64)

    perm = _qk_perm()
    wq_l = _fp8(_dr_pack((SW * Wq)[perm]))
    wk_l = _fp8(_dr_pack((SW * Wk)[perm]))
    bqP = (SW * bq)[perm]
    bkP = (SW * bk)[perm]
    # V: [vb][t][p][(j, n)]: SW * Wv[vb*512+n, (2t+j)*128+p]
    WvS = (SW * Wv).reshape(2, 512, NCK, P)     # [vb, n, kin, p]
    wv_l = np.empty((2, 4, P, 2, 512), np.float64)
    for t in range(4):
        for j in range(2):
            wv_l[:, t, :, j, :] = WvS[:, :, 2 * t + j, :].transpose(0, 2, 1)
    wv_l = _fp8(wv_l.reshape(2, 4, P, 1024))
    wm_l = _fp8(_dr_pack(SW * Wm))
    w1_l = _bf16(W1.reshape(NFF, P, NCK, P).transpose(0, 3, 2, 1)
                 .reshape(NFF, P, C))
    w2_l = _bf16(W2.reshape(NCK, P, NFF, P).transpose(0, 3, 2, 1)
                 .reshape(NCK, P, FF))
    bqk_l = _f32(np.concatenate([bqP.reshape(NCK, P).T,
                                 bkP.reshape(NCK, P).T], axis=1))
    bm_l = _f32(bm.reshape(NCK, P).T)
    b1_l = _f32(b1.reshape(NFF, P).T)
    b2_l = _f32(b2.reshape(NCK, P).T)

    shared = dict(wq=wq_l, wk=wk_l, wv=wv_l, wm=wm_l, w1=w1_l, w2=w2_l,
                  bqk=bqk_l, bm=bm_l, b1=b1_l, b2=b2_l)

    in_maps = []
    for core in range(8):
        b_, m = divmod(core, 2)
        xb = x[b_]
        xp = np.concatenate([xb[m * MT:(m + 1) * MT],
                             xb[(1 - m) * MT:(2 - m) * MT]], axis=0)
        xt_l = _bf16(xp.reshape(NT, NCK, P).transpose(2, 1, 0)
                     .reshape(P, NCK * NT))
        in_maps.append(dict(shared, xt=xt_l))
    return in_maps


def gather_output(results):
    out = np.empty((B, N, C), np.float32)
    for core in range(8):
        b_, m = divmod(core, 2)
        O = np.asarray(results[core]["outT"]).astype(np.float32)
        O = O.reshape(P, NCK, MT)
        out[b_, m * MT:(m + 1) * MT, :] = O.transpose(2, 1, 0).reshape(MT, C)
    return out


_CACHE = {}


def kernel(**inputs):
    if "nc" not in _CACHE:
        _CACHE["nc"] = build_module()
    nc = _CACHE["nc"]
    in_maps = prepare_inputs(**inputs)
    res = run_bass_kernel_spmd(nc, in_maps, core_ids=list(range(8)))
    return gather_output(res.results)
